# revision 1
# baseline (speedup 1.0000x reference)
"""Trainium2 Bass kernel for ContractiveInvertibleGNN feed-forward.

Math (reference, with group_mask == I_32):
  out[b,i] = f_i( sum_j W_adj[j,i] * g_j(X[b,j]) )
where g_j: R -> R^32 and f_i: R^32 -> R are slices of two shared MLPs
(64->128->128->32 with a residual middle block, LeakyReLU 0.01):
  g: H1 = lrelu(X[b,j]*U_j + C1_j); H2 = H1 + lrelu(H1@W2g + b2g)
     X_emb = H2 @ W3g + b3g
  f: in = [X_aggr ; emb_i] -> Hf1 = lrelu(X_aggr@Wf1x + C2_i)
     Hf2 = Hf1 + lrelu(Hf1@Wf2 + bf2); out_i = Hf2 . V_i (+ bf3_i)
with per-node constants U_j = g_W1[j,:], C1_j = emb_j@g_W1[32:]+g_b1,
C2_i = emb_i@f_W1[32:]+f_b1 (+ (sum_j W_adj[j,i])*g_b3@f_W1[:32]),
V_i = f_W3[:,i].

Sharding: pure data-parallel over batch across 8 cores (2048 rows each).

On-chip layout (per core): node-major columns. g-phase runs per node j over
[128, 2048] tiles; X_emb assembled as Xe[(c,d), (j,t)] with c = batch
quarter stacked on partition groups; StreamTranspose -> Xt[(c,j),(t,d)];
block-diag(W_adj) matmul aggregates over j; StreamTranspose back ->
Xa[(c,d),(i,t)]; f-phase per node i with padded stationaries selecting
partition group c; final dot with V_i via a [128,4] stationary that also
routes batch quarter c to psum row c.
"""

import os
import sys

import numpy as np

for _p in ("/opt/trn_rl_repo", "/root/.axon_site/_ro/trn_rl_repo"):
    if os.path.isdir(_p) and _p not in sys.path:
        sys.path.insert(0, _p)

N = 32          # nodes
D = 32          # processed dim (== N, group_mask = I)
A = 128         # hidden width
B = 16384       # batch
NCORES = 8
BC = B // NCORES        # 2048 rows per core
CH = 512                # matmul free-dim chunk
NCH = BC // CH          # 4 chunks (partition-group stacking factor)
ALPHA = 0.01

_F32R = None  # set lazily to mybir.dt.float32r


def _build_program(zero_b2=True):
    from contextlib import ExitStack

    from concourse import bacc, bass, mybir, tile

    global _F32R
    _F32R = mybir.dt.float32r
    f32 = mybir.dt.float32
    LRELU = mybir.ActivationFunctionType.Lrelu
    ALU_MULT = mybir.AluOpType.mult
    ALU_ADD = mybir.AluOpType.add
    ALU_MAX = mybir.AluOpType.max

    nc = bacc.Bacc("TRN2", target_bir_lowering=False, debug=False)

    f32r = mybir.dt.float32r

    def din(name, shape, dt=None):
        return nc.dram_tensor(
            name, list(shape), dt or f32r, kind="ExternalInput"
        ).ap()

    xt_d = din("XT", (N, BC), f32)
    gw2_d = din("GW2", (A, A))
    fw2_d = din("FW2", (A, A))
    gw3p_d = din("GW3P", (A, NCH * A))     # col-block c: rows of g_W3 at M cols 32c..
    fw1p_d = din("FW1P", (A, NCH * A))     # row-block c: f_W1[:32] at K rows 32c..
    bd_d = din("BD", (A, A))               # kron(I4, W_adj)
    u_d = din("U", (A, N), f32)
    c1_d = din("C1", (A, N), f32)
    c2_d = din("C2", (A, N), f32)
    gb2_d = din("GB2", (A, 1), f32)
    fb2_d = din("FB2", (A, 1), f32)
    vp_d = din("VP", (A, (N + 1) * D))     # [:, (i+1)*D] = V_i[a]; else 0
    out_d = nc.dram_tensor("OUT", [N, BC], f32, kind="ExternalOutput").ap()

    with tile.TileContext(nc) as tc, ExitStack() as ctx:
        const = ctx.enter_context(tc.tile_pool(name="const", bufs=1))
        bigp = ctx.enter_context(tc.tile_pool(name="big", bufs=2))
        workp = ctx.enter_context(tc.tile_pool(name="work", bufs=6))
        outp = ctx.enter_context(tc.tile_pool(name="outs", bufs=2))
        lrp = ctx.enter_context(tc.tile_pool(name="lrp", bufs=3))
        ppA = ctx.enter_context(tc.tile_pool(name="ppA", bufs=2, space="PSUM"))
        ppB = ctx.enter_context(tc.tile_pool(name="ppB", bufs=2, space="PSUM"))
        ppR = ctx.enter_context(tc.tile_pool(name="ppR", bufs=2, space="PSUM"))

        def load_const(ap_dram, shape):
            t = const.tile(list(shape), ap_dram.dtype,
                           tag=f"c_{ap_dram.tensor.name}")
            nc.sync.dma_start(t[:, :], ap_dram)
            return t

        gw2_s = load_const(gw2_d, (A, A))
        fw2_s = load_const(fw2_d, (A, A))
        gw3p_s = load_const(gw3p_d, (A, NCH * A))
        fw1p_s = load_const(fw1p_d, (A, NCH * A))
        bd_s = load_const(bd_d, (A, A))
        u_s = load_const(u_d, (A, N))
        c1_s = load_const(c1_d, (A, N))
        c2_s = load_const(c2_d, (A, N))
        gb2_s = load_const(gb2_d, (A, 1))
        fb2_s = load_const(fb2_d, (A, 1))
        vp_s = load_const(vp_d, (A, (N + 1) * D))


        # Xe[(c,d), (j,t)] = X_emb[d, j, c*CH+t]
        xe = bigp.tile([A, N * CH], f32r, tag="big")

        # ---------------- g phase: one node j per iteration ----------------
        for j in range(N):
            xbc = workp.tile([A, BC], f32, tag="w")
            nc.sync.dma_start(
                xbc[:, :], xt_d[j : j + 1, :].partition_broadcast(A)
            )
            h1 = workp.tile([A, BC], f32r, tag="w")
            SPL = 3 * CH
            nc.scalar.activation(
                h1[:, :SPL], xbc[:, :SPL], LRELU,
                bias=c1_s[:, j : j + 1], scale=u_s[:, j : j + 1], alpha=ALPHA,
            )
            zt = lrp.tile([A, CH], f32, tag="z")
            mt = lrp.tile([A, CH], f32, tag="m")
            nc.vector.tensor_scalar(zt[:, :], xbc[:, SPL:],
                                    u_s[:, j : j + 1], c1_s[:, j : j + 1],
                                    ALU_MULT, ALU_ADD)
            nc.vector.tensor_scalar(mt[:, :], zt[:, :], ALPHA, None, ALU_MULT)
            nc.vector.tensor_tensor(h1[:, SPL:], zt[:, :], mt[:, :], ALU_MAX)
            t2 = workp.tile([A, BC], f32r, tag="w")
            pm3 = ppB.tile([A, CH], f32, tag="pB")
            for h in range(2):  # halves of 1024 cols
                pa = ppA.tile([A, 2 * CH], f32, tag="pA")
                for q in range(2):
                    sl = slice(h * 2 * CH + q * CH, h * 2 * CH + (q + 1) * CH)
                    nc.tensor.matmul(
                        pa[:, q * CH : (q + 1) * CH], gw2_s[:, :],
                        h1[:, sl], start=True, stop=True,
                    )
                nc.scalar.activation(
                    t2[:, h * 2 * CH : (h + 1) * 2 * CH], pa[:, :], LRELU,
                    bias=gb2_s[:, 0:1], alpha=ALPHA,
                )
            # X_emb = g_W3^T @ (H1 + lrelu(.)) via 8 accumulating matmuls,
            # chunk c routed to psum rows 32c by the padded stationary.
            for c in range(NCH):
                lt = gw3p_s[:, c * A : (c + 1) * A]
                sl = slice(c * CH, (c + 1) * CH)
                nc.tensor.matmul(pm3[:, :], lt, h1[:, sl],
                                 start=(c == 0), stop=False)
                nc.tensor.matmul(pm3[:, :], lt, t2[:, sl],
                                 start=False, stop=(c == NCH - 1))
            nc.vector.tensor_copy(xe[:, j * CH : (j + 1) * CH], pm3[:, :])

        # ---------------- aggregation ----------------
        # T1: Xe[(c,d),(j,t)] -> Xt[(c,j),(t,d)]
        xt3 = xe.bitcast(f32).rearrange(
            "p (j t) -> p j t", j=N).transpose([0, 2, 1])
        xtile = bigp.tile([A, CH * D], f32, tag="big")
        xto = xtile.rearrange("p (t d) -> p t d", d=D)
        TS = 8  # split into 8 ops for overlap
        tstep = CH // TS
        for s in range(TS):
            nc.vector.transpose(
                xto[:, s * tstep : (s + 1) * tstep, :],
                xt3[:, s * tstep : (s + 1) * tstep, :],
            )
        # DMA hop: rounded-bits copy into an f32r-typed tensor for the PE
        xtile_r = bigp.tile([A, CH * D], f32r, tag="big")
        for s in range(TS):
            sl = slice(s * (CH * D // TS), (s + 1) * (CH * D // TS))
            nc.sync.dma_start(xtile_r[:, sl], xtile.bitcast(f32r)[:, sl])
        # agg windows + T2-back: psum[(c,i),(t16,d)] -> Xa[(c,d),(i,t)]
        xa = bigp.tile([A, N * CH], f32, tag="big")
        xa3 = xa.rearrange("p (i t) -> p i t", i=N).transpose([0, 2, 1])
        WT = CH // D  # 16 t per window
        for w in range(CH // WT):  # 32 windows
            pg = ppB.tile([A, CH], f32, tag="pB")
            nc.tensor.matmul(
                pg[:, :], bd_s[:, :],
                xtile_r[:, w * CH : (w + 1) * CH], start=True, stop=True,
            )
            nc.vector.transpose(
                xa3[:, w * WT : (w + 1) * WT, :],
                pg.rearrange("p (t d) -> p t d", d=D)[:, :, :],
            )

        xa_r = bigp.tile([A, N * CH], f32r, tag="big")
        for s in range(TS):
            sl = slice(s * (N * CH // TS), (s + 1) * (N * CH // TS))
            nc.sync.dma_start(xa_r[:, sl], xa.bitcast(f32r)[:, sl])

        # ---------------- f phase: one node i per iteration ----------------
        for i in range(N):
            rhs = xa_r[:, i * CH : (i + 1) * CH]
            hf1 = workp.tile([A, BC], f32r, tag="w")
            tf = workp.tile([A, BC], f32r, tag="w")
            for h in range(2):
                pa = ppA.tile([A, 2 * CH], f32, tag="pA")
                for q in range(2):
                    c = h * 2 + q
                    nc.tensor.matmul(
                        pa[:, q * CH : (q + 1) * CH],
                        fw1p_s[:, c * A : (c + 1) * A], rhs,
                        start=True, stop=True,
                    )
                nc.scalar.activation(
                    hf1[:, h * 2 * CH : (h + 1) * 2 * CH], pa[:, :], LRELU,
                    bias=c2_s[:, i : i + 1], alpha=ALPHA,
                )
            for h in range(2):
                pa = ppA.tile([A, 2 * CH], f32, tag="pA")
                for q in range(2):
                    c = h * 2 + q
                    nc.tensor.matmul(
                        pa[:, q * CH : (q + 1) * CH], fw2_s[:, :],
                        hf1[:, c * CH : (c + 1) * CH], start=True, stop=True,
                    )
                nc.scalar.activation(
                    tf[:, h * 2 * CH : (h + 1) * 2 * CH], pa[:, :], LRELU,
                    bias=fb2_s[:, 0:1], alpha=ALPHA,
                )
            pr = ppR.tile([D, CH], f32, tag="pR")
            for c in range(NCH):
                base = (i + 1) * D - c
                lt = vp_s[:, base : base + D]
                nc.tensor.matmul(pr[:, :], lt, hf1[:, c * CH : (c + 1) * CH],
                                 start=(c == 0), stop=False)
                nc.tensor.matmul(pr[:, :], lt, tf[:, c * CH : (c + 1) * CH],
                                 start=False, stop=(c == NCH - 1))
            osb = outp.tile([NCH, CH], f32, tag="o")
            nc.vector.tensor_copy(osb[:, :], pr[:NCH, :])
            nc.sync.dma_start(
                out_d[i : i + 1, :].rearrange("o (c t) -> (o c) t", c=NCH),
                osb[:, :],
            )

    nc.compile()
    return nc


_NC_CACHE = {}


def _get_program(zero_b2=True):
    if zero_b2 not in _NC_CACHE:
        _NC_CACHE[zero_b2] = _build_program(zero_b2)
    return _NC_CACHE[zero_b2]


def _host_consts(W, embeddings, g_W1, g_b1, g_W2, g_b2, g_W3, g_b3,
                 f_W1, f_b1, f_W2, f_b2, f_W3, f_b3):
    f = np.float32
    W_adj = (W * (1.0 - np.eye(N, dtype=f))).astype(f)
    U = np.ascontiguousarray(g_W1[:D].T, dtype=f)                    # [A, N]
    C1 = np.ascontiguousarray((embeddings @ g_W1[D:] + g_b1).T, f)   # [A, N]
    s = W_adj.sum(axis=0)                                            # [N]
    C2 = (embeddings @ f_W1[D:] + f_b1 + np.outer(s, g_b3 @ f_W1[:D]))
    C2 = np.ascontiguousarray(C2.T, dtype=f)                         # [A, N]
    GW3P = np.zeros((A, NCH * A), f)
    FW1P = np.zeros((A, NCH * A), f)
    for c in range(NCH):
        GW3P[:, c * A + c * D : c * A + (c + 1) * D] = g_W3
        FW1P[c * D : (c + 1) * D, c * A : (c + 1) * A] = f_W1[:D]
    BD = np.kron(np.eye(NCH, dtype=f), W_adj).astype(f)
    VP = np.zeros((A, (N + 1) * D), f)
    for i in range(N):
        VP[:, (i + 1) * D] = f_W3[:, i]
    return {
        "GW2": np.ascontiguousarray(g_W2, f),
        "FW2": np.ascontiguousarray(f_W2, f),
        "GW3P": GW3P, "FW1P": FW1P, "BD": BD,
        "U": U, "C1": C1, "C2": C2,
        "GB2": np.ascontiguousarray(g_b2.reshape(A, 1), f),
        "FB2": np.ascontiguousarray(f_b2.reshape(A, 1), f),
        "VP": VP,
    }


def _kernel_numpy(X, W, embeddings, g_W1, g_b1, g_W2, g_b2, g_W3, g_b3,
                  f_W1, f_b1, f_W2, f_b2, f_W3, f_b3, group_mask):
    # general fallback (non-identity group_mask)
    def lrelu(x):
        return np.where(x > 0, x, ALPHA * x)

    def mlp(x, W1, b1, W2, b2, W3, b3):
        h = lrelu(x @ W1 + b1)
        h = h + lrelu(h @ W2 + b2)
        return h @ W3 + b3

    n = W.shape[0]
    W_adj = W * (1.0 - np.eye(n, dtype=W.dtype))
    Xm = X[:, None, :] * group_mask
    E = np.broadcast_to(embeddings, (X.shape[0], n, embeddings.shape[1]))
    Xe = mlp(np.concatenate([Xm, E], 2), g_W1, g_b1, g_W2, g_b2, g_W3, g_b3)
    Xa = np.einsum("ji,bjd->bid", W_adj, Xe)
    Xr = mlp(np.concatenate([Xa, E], 2), f_W1, f_b1, f_W2, f_b2, f_W3, f_b3)
    return (Xr * group_mask).sum(axis=1).astype(np.float32)


def kernel(X, W, embeddings, g_W1, g_b1, g_W2, g_b2, g_W3, g_b3,
           f_W1, f_b1, f_W2, f_b2, f_W3, f_b3, group_mask, _run_kw=None):
    if not np.allclose(group_mask, np.eye(N, D, dtype=np.float32)):
        return _kernel_numpy(X, W, embeddings, g_W1, g_b1, g_W2, g_b2, g_W3,
                             g_b3, f_W1, f_b1, f_W2, f_b2, f_W3, f_b3,
                             group_mask)

    from concourse import bass_utils

    consts = _host_consts(W, embeddings, g_W1, g_b1, g_W2, g_b2, g_W3, g_b3,
                          f_W1, f_b1, f_W2, f_b2, f_W3, f_b3)
    XT = np.ascontiguousarray(np.asarray(X, np.float32).T)  # [N, B]
    in_maps = []
    for k in range(NCORES):
        m = dict(consts)
        m["XT"] = np.ascontiguousarray(XT[:, k * BC : (k + 1) * BC])
        in_maps.append(m)

    nc = _get_program()
    res = bass_utils.run_bass_kernel_spmd(
        nc, in_maps, core_ids=list(range(NCORES)), **(_run_kw or {})
    )
    out = np.empty((B, D), np.float32)
    for k in range(NCORES):
        out[k * BC : (k + 1) * BC, :] = res.results[k]["OUT"].T
    out += f_b3.reshape(1, D).astype(np.float32)
    if _run_kw:
        kernel.last_results = res
    return out



# revision 79
# speedup vs baseline: 1.4376x; 1.4376x over previous
"""Trainium2 Bass kernel for ContractiveInvertibleGNN feed-forward.

Math (reference, with group_mask == I_32):
  out[b,i] = f_i( sum_j W_adj[j,i] * g_j(X[b,j]) )
where g_j: R -> R^32 and f_i: R^32 -> R are slices of two shared MLPs
(64->128->128->32 with a residual middle block, LeakyReLU 0.01):
  g: H1 = lrelu(X[b,j]*U_j + C1_j); H2 = H1 + lrelu(H1@W2g + b2g)
     X_emb = H2 @ W3g            (g_b3 folded into C2)
  f: Hf1 = lrelu(X_aggr@Wf1x + C2_i)
     Hf2 = Hf1 + lrelu(Hf1@Wf2 + bf2); out_i = Hf2 . V_i (+ f_b3_i on host)

Sharding: pure data-parallel over batch across 8 cores (2048 rows each).

Per-core schedule: batch is processed in 2 halves of 1024, emitted as a
software pipeline (per-engine execution follows emission order):
g(h0); then g(h1) interleaved with the agg of h0 (T1 transpose ->
block-diag matmul -> T2 transpose) and the first f(h0) nodes; then the
rest of f(h0) interleaved with agg(h1); then f(h1). gw3 / V-dot matmuls
are emitted with a 1-2 node skew so the PE never waits on the current
node's activations.

Most SBUF interchange tiles are bf16 (DVE 2x/4x modes, 1 cyc/row PE,
half DMA); PSUM stays f32. HW constraints honored: GPSIMD never touches
PSUM; StreamTranspose src/dst dtypes match (so Xa stays f32 and is
re-rounded to f32r for the PE by small SBUF->SBUF DMA hops); no mixed
32/16-bit matmul inputs. Engine assignment: Act = psum-sourced lrelus
(t2/hf1/tf) + a per-phase share of h1; DVE = most of the h1 lrelu
(tensor_scalar 4x path), transposes, psum->bf16 copies, tf tail, and a
slice of the f-residual add; Pool = SBUF-only work (h1 tail, most of
the f-residual add); PE = matmuls with the g-residual folded as double
accumulating matmuls. Final dot V_i routes node i / quarter c to PSUM
partition 4i+c via a padded stationary table (VPX), giving one
[128, 256] output copy + one DMA per half.
"""

import os
import sys

import numpy as np

for _p in ("/opt/trn_rl_repo", "/root/.axon_site/_ro/trn_rl_repo"):
    if os.path.isdir(_p) and _p not in sys.path:
        sys.path.insert(0, _p)

N = 32          # nodes
D = 32          # processed dim (== N, group_mask = I)
A = 128         # hidden width
B = 16384       # batch
NCORES = 8
BC = B // NCORES        # 2048 rows per core
NH = 2                  # batch halves per core
BH = BC // NH           # 1024 rows per half
NCH = 4                 # partition-group (quarter) stacking factor
CH = BH // NCH          # 256 cols per quarter
ALPHA = 0.01

# knobs
TG_POOL = 0             # t2 cols per half on Pool engine (rest on Act)
PF_DVE = 448            # tf cols per half on DVE (rest on Act)
VREG = A + 3            # per-node stationary region width in VPX


def _build_program():
    from contextlib import ExitStack

    from concourse import bacc, bass, mybir, tile

    f32 = mybir.dt.float32
    f32r = mybir.dt.float32r
    bf16 = mybir.dt.bfloat16
    LRELU = mybir.ActivationFunctionType.Lrelu
    ALU_MULT = mybir.AluOpType.mult
    ALU_ADD = mybir.AluOpType.add
    ALU_MAX = mybir.AluOpType.max

    nc = bacc.Bacc("TRN2", target_bir_lowering=False, debug=False)

    def din(name, shape, dt):
        return nc.dram_tensor(
            name, list(shape), dt, kind="ExternalInput"
        ).ap()

    xtb_d = din("XTB", (N, BC), bf16)
    gw2_d = din("GW2", (A, A), bf16)
    fw2_d = din("FW2", (A, A), bf16)
    gw3p_d = din("GW3P", (A, NCH * A), bf16)   # col-block c: g_W3 at cols 32c..
    fw1p_d = din("FW1P", (A, NCH * A), bf16)   # row-block c: f_W1[:32] rows 32c
    bd_d = din("BD", (A, A), bf16)             # kron(I4, W_adj)
    vpx_d = din("VPX", (A, N * VREG), bf16)    # region i: V_i at col 131i+4i+3
    u_d = din("U", (A, N), f32)
    ua_d = din("UA", (A, N), f32)
    c1_d = din("C1", (A, N), f32)
    c1a_d = din("C1A", (A, N), f32)
    c2_d = din("C2", (A, N), f32)
    gb2_d = din("GB2", (A, 1), f32)
    fb2_d = din("FB2", (A, 1), f32)
    out_d = nc.dram_tensor("OUT", [N, BC], f32, kind="ExternalOutput").ap()

    with tile.TileContext(nc) as tc, ExitStack() as ctx:
        const = ctx.enter_context(tc.tile_pool(name="const", bufs=1))
        bigp = ctx.enter_context(tc.tile_pool(name="big", bufs=2))
        workp = ctx.enter_context(tc.tile_pool(name="work", bufs=5))
        xbcp = ctx.enter_context(tc.tile_pool(name="xbc", bufs=5))
        lrp = ctx.enter_context(tc.tile_pool(name="lrp", bufs=2))
        outp = ctx.enter_context(tc.tile_pool(name="outs", bufs=2))
        xarp = ctx.enter_context(tc.tile_pool(name="xar", bufs=3))
        # PSUM bank budget (8 banks of 2KB): ppG 3 + ppF 4 + ppV 1.
        # ppG serves gw2-out, gw3-out pairs and agg windows; ppF serves
        # the f phase only, so g(h+1) never waits on f(h) ring slots.
        ppG = ctx.enter_context(tc.tile_pool(name="ppG", bufs=3, space="PSUM"))
        ppF = ctx.enter_context(tc.tile_pool(name="ppF", bufs=4, space="PSUM"))
        ppV = ctx.enter_context(tc.tile_pool(name="ppV", bufs=1, space="PSUM"))

        def load_const(ap_dram, shape):
            t = const.tile(list(shape), ap_dram.dtype,
                           tag=f"c_{ap_dram.tensor.name}")
            nc.sync.dma_start(t[:, :], ap_dram)
            return t

        st = [dict() for _ in range(NH)]
        u_s = load_const(u_d, (A, N))
        ua_s = load_const(ua_d, (A, N))
        c1_s = load_const(c1_d, (A, N))
        c1a_s = load_const(c1a_d, (A, N))
        gw2_s = load_const(gw2_d, (A, A))
        gb2_s = load_const(gb2_d, (A, 1))
        heavy = {}
        heavy["gw3p"] = load_const(gw3p_d, (A, NCH * A))
        heavy["bd"] = load_const(bd_d, (A, A))
        heavy["fw1p"] = load_const(fw1p_d, (A, NCH * A))
        heavy["fw2"] = load_const(fw2_d, (A, A))
        heavy["c2"] = load_const(c2_d, (A, N))
        heavy["fb2"] = load_const(fb2_d, (A, 1))

        # Per-half state for the software-pipelined emission below.
        TSP = BH - TG_POOL
        SPL = BH - PF_DVE
        TS = 8                  # T1 transpose steps per half
        WT = 512 // D           # 16 t per agg window
        NW = CH * D // 512      # 16 agg windows per half
        def g_node(h, j, h1_act=False, gpool=None):
            """Emit g work for node j; gw3+copy are skewed (emitted for
            j-1) so the PE never stalls on the current node's t2. The h1
            lrelu runs on Act or DVE per the h1_act flag, chosen by the
            emission schedule to balance per-phase engine load."""
            s = st[h]
            if j == 0:
                s["xe"] = bigp.tile([A, N * CH], bf16, tag="xe", name="xe")
                s["gq"] = {}
            if j in s.get("pre", {}):
                xbc = s["pre"].pop(j)
            else:
                xbc = xbcp.tile([A, BH], bf16, tag="xbc")
                nc.sync.dma_start(
                    xbc[:, :],
                    xtb_d[j : j + 1,
                          h * BH : (h + 1) * BH].partition_broadcast(A),
                )
            h1 = workp.tile([A, BH], bf16, tag="h1")
            uj = u_s[:, j : j + 1]
            cj = c1_s[:, j : j + 1]
            uaj = ua_s[:, j : j + 1]
            caj = c1a_s[:, j : j + 1]
            if h1_act:
                nc.scalar.activation(h1[:, :], xbc[:, :], LRELU,
                                     bias=cj, scale=uj, alpha=ALPHA)
            else:
                GP = 192  # h1 tail cols on the otherwise-idle Pool engine
                zt = lrp.tile([A, BH], bf16, tag="z")
                mt = lrp.tile([A, BH], bf16, tag="m")
                nc.vector.tensor_scalar(zt[:, : BH - GP], xbc[:, : BH - GP],
                                        uj, cj, ALU_MULT, ALU_ADD)
                nc.vector.tensor_scalar(mt[:, : BH - GP], xbc[:, : BH - GP],
                                        uaj, caj, ALU_MULT, ALU_ADD)
                nc.vector.tensor_tensor(h1[:, : BH - GP], zt[:, : BH - GP],
                                        mt[:, : BH - GP], ALU_MAX)
                nc.gpsimd.tensor_scalar(zt[:, BH - GP :], xbc[:, BH - GP :],
                                        uj, cj, ALU_MULT, ALU_ADD)
                nc.gpsimd.tensor_scalar(mt[:, BH - GP :], xbc[:, BH - GP :],
                                        uaj, caj, ALU_MULT, ALU_ADD)
                nc.vector.tensor_tensor(h1[:, BH - GP :], zt[:, BH - GP :],
                                        mt[:, BH - GP :], ALU_MAX)
            # middle residual block: t2 = lrelu(h1 @ gw2 + gb2)
            t2 = workp.tile([A, BH], bf16, tag="t2")
            for q in range(2):
                pa = (gpool or ppG).tile(
                    [A, 512], f32,
                    tag="pF" if gpool is ppF else "pG", name="pa")
                sl = slice(q * 512, (q + 1) * 512)
                nc.tensor.matmul(pa[:, :], gw2_s[:, :], h1[:, sl],
                                 start=True, stop=True)
                if q == 0:
                    nc.scalar.activation(t2[:, sl], pa[:, :], LRELU,
                                         bias=gb2_s[:, 0:1], alpha=ALPHA)
                else:
                    nc.scalar.activation(
                        t2[:, 512 : TSP], pa[:, : TSP - 512], LRELU,
                        bias=gb2_s[:, 0:1], alpha=ALPHA,
                    )
                    if TG_POOL:
                        gz = lrp.tile([A, TG_POOL], bf16, tag="gz")
                        nc.gpsimd.tensor_scalar(gz[:, :], pa[:, TSP - 512 :],
                                                ALPHA, None, ALU_MULT)
                        nc.gpsimd.tensor_tensor(t2[:, TSP:],
                                                pa[:, TSP - 512 :],
                                                gz[:, :], ALU_MAX)
            s["gq"][j] = (h1, t2)
            # skew: emit X_emb projection for the pair (j-3, j-2) so the
            # PE never stalls on the current node's t2.
            if j >= 3 and j % 2 == 1:
                g_emb(h, j - 3)

        def g_emb(h, j):
            """X_emb for nodes j and j+1 into one PSUM tile: gw3^T @
            (h1 + t2) with the residual folded as double accumulating
            matmuls; quarter c routed to psum rows 32c."""
            s = st[h]
            pm3_full = ppG.tile([A, 512], f32, tag="pG", name="pm3")
            for u in range(2):
                h1, t2 = s["gq"].pop(j + u)
                pm3 = pm3_full[:, u * CH : (u + 1) * CH]
                for c in range(NCH):
                    lt = heavy["gw3p"][:, c * A : (c + 1) * A]
                    csl = slice(c * CH, (c + 1) * CH)
                    nc.tensor.matmul(pm3[:, :], lt, h1[:, csl],
                                     start=(c == 0), stop=False)
                    nc.tensor.matmul(pm3[:, :], lt, t2[:, csl],
                                     start=False, stop=(c == NCH - 1))
            nc.vector.tensor_copy(s["xe"][:, j * CH : (j + 2) * CH],
                                  pm3_full[:, :])

        def agg_step(h, k):
            """Step k of the aggregation pipeline: 0..TS-1 are T1 block
            transposes, TS..TS+NW-1 are (block-diag matmul, T2) pairs."""
            s = st[h]
            if k == 0:
                s["xtile"] = bigp.tile([A, CH * D], bf16, tag="xt",
                                       name="xtile", bufs=1)
                s["xa"] = bigp.tile([A, N * CH], f32, tag="xa", name="xa")
            xe, xtile, xa = s["xe"], s["xtile"], s["xa"]
            if k < TS:
                xt3 = xe.rearrange("p (j t) -> p j t",
                                   j=N).transpose([0, 2, 1])
                xto = xtile.rearrange("p (t d) -> p t d", d=D)
                tstep = CH // TS
                nc.vector.transpose(
                    xto[:, k * tstep : (k + 1) * tstep, :],
                    xt3[:, k * tstep : (k + 1) * tstep, :],
                )
                return
            w = k - TS
            xa3 = xa.rearrange("p (i t) -> p i t", i=N).transpose([0, 2, 1])
            pg = ppG.tile([A, 512], f32, tag="pG", name="pg")
            nc.tensor.matmul(
                pg[:, :], heavy["bd"][:, :],
                xtile[:, w * 512 : (w + 1) * 512], start=True, stop=True,
            )
            nc.vector.transpose(
                xa3[:, w * WT : (w + 1) * WT, :],
                pg.rearrange("p (t d) -> p t d", d=D)[:, :, :],
            )

        def f_hop(h, i):
            # rounded-bits SBUF->SBUF DMA: xa (f32) -> f32r for the PE,
            # two nodes per transfer
            s = st[h]
            xr = xarp.tile([A, 2 * CH], f32r, tag="xr", name="xr")
            nc.sync.dma_start(
                xr[:, :], s["xa"].bitcast(f32r)[:, i * CH : (i + 2) * CH])
            s["hop"][i] = xr

        def f_node(h, i):
            """Emit f work for node i; the V-dot is skewed (emitted for
            i-1) so the PE never stalls on the current node's hf2."""
            s = st[h]
            if i == 0:
                s["vps"] = ppV.tile([A, CH], f32, tag="pV", name="vps")
                s["fq"] = {}
                s["hop"] = {}
                f_hop(h, 0)
                f_hop(h, 2)
                f_hop(h, 4)
            if i % 2 == 0 and i + 6 < N:
                f_hop(h, i + 6)
            xr = s["hop"].pop(i - 1) if i % 2 else s["hop"][i]
            rhs = xr[:, (i % 2) * CH : (i % 2 + 1) * CH]
            hf1 = workp.tile([A, BH], bf16, tag="hf1", bufs=3)
            for q in range(2):
                paf = ppF.tile([A, 512], f32, tag="pF", name="paf")
                for cc in range(2):
                    c = 2 * q + cc
                    nc.tensor.matmul(
                        paf[:, cc * CH : (cc + 1) * CH],
                        heavy["fw1p"][:, c * A : (c + 1) * A], rhs,
                        start=True, stop=True,
                    )
                sl = slice(q * 512, (q + 1) * 512)
                nc.scalar.activation(hf1[:, sl], paf[:, :], LRELU,
                                     bias=heavy["c2"][:, i : i + 1],
                                     alpha=ALPHA)
                # skew: V-dots from three nodes back fill the PE while
                # this node's activations run.
                if q == 0 and i > 2:
                    f_vdot(h, i - 3)
            tf = workp.tile([A, BH], bf16, tag="tf", bufs=3)
            for q in range(2):
                pbf = ppF.tile([A, 512], f32, tag="pF", name="pbf")
                sl = slice(q * 512, (q + 1) * 512)
                for cc in range(2):
                    c = 2 * q + cc
                    csl = slice(c * CH, (c + 1) * CH)
                    nc.tensor.matmul(pbf[:, cc * CH : (cc + 1) * CH],
                                     heavy["fw2"][:, :], hf1[:, csl],
                                     start=True, stop=True)
                if q == 0:
                    nc.scalar.activation(tf[:, sl], pbf[:, :], LRELU,
                                         bias=heavy["fb2"][:, 0:1],
                                         alpha=ALPHA)
                else:
                    asl = 512 - (PF_DVE if h == 0 else 512)
                    if asl:
                        nc.scalar.activation(tf[:, 512 : 512 + asl],
                                             pbf[:, :asl], LRELU,
                                             bias=heavy["fb2"][:, 0:1],
                                             alpha=ALPHA)
                    dz = lrp.tile([A, 512], bf16, tag="dz")
                    dz = dz[:, : 512 - asl]
                    nc.vector.tensor_scalar(dz[:, :], pbf[:, asl:],
                                            ALPHA, None, ALU_MULT)
                    nc.vector.tensor_tensor(tf[:, 512 + asl :],
                                            pbf[:, asl:], dz[:, :],
                                            ALU_MAX)
            hf2 = workp.tile([A, BH], bf16, tag="hf2", bufs=5)
            # residual add split across the SBUF-only Pool engine and DVE
            nc.gpsimd.tensor_tensor(hf2[:, :640], hf1[:, :640],
                                    tf[:, :640], ALU_ADD)
            nc.vector.tensor_tensor(hf2[:, 640:], hf1[:, 640:],
                                    tf[:, 640:], ALU_ADD)
            s["fq"][i] = hf2

        def f_vdot(h, i):
            # out_i = hf2 . V_i via accumulating matmuls; stationary
            # window puts V_i at psum partition 4i+c.
            s = st[h]
            hf2 = s["fq"].pop(i)
            for c in range(NCH):
                base = VREG * i + 3 - c
                lt = heavy["vpx"][:, base : base + A]
                sl = slice(c * CH, (c + 1) * CH)
                nc.tensor.matmul(
                    s["vps"][:, :], lt, hf2[:, sl],
                    start=(i == 0 and c == 0),
                    stop=(i == N - 1 and c == NCH - 1),
                )

        def f_out(h):
            f_vdot(h, N - 3)
            f_vdot(h, N - 2)
            f_vdot(h, N - 1)
            osb = outp.tile([A, CH], f32, tag="o")
            nc.vector.tensor_copy(osb[:, :], st[h]["vps"][:, :])
            nc.gpsimd.dma_start(
                out_d[:, h * BH : (h + 1) * BH].rearrange(
                    "i (c t) -> i c t", c=NCH),
                osb[:, :],
            )

        # ---- software-pipelined emission across the two halves ----
        NAGG = TS + NW  # 20 agg steps per half
        for j in range(N):
            g_node(0, j, h1_act=(j % 4 == 3))
            if j == 0:
                heavy["vpx"] = load_const(vpx_d, (A, N * VREG))
        g_emb(0, N - 2)
        # agg(0) interleaved with the first 20 nodes of g(1); the last 12
        # nodes of g(1) interleave 1:1 with the first 12 nodes of f(0).
        fi = 0
        for k in range(N):
            g_node(1, k, h1_act=(k < NAGG and k % 3 == 0),
                   gpool=(ppF if k < NAGG - 2 else None))
            if k < NAGG:
                agg_step(0, k)
            else:
                f_node(0, fi)
                fi += 1
        g_emb(1, N - 2)
        # remaining f(0) nodes interleave with agg(1)
        for k in range(NAGG):
            f_node(0, fi)
            fi += 1
            agg_step(1, k)
        while fi < N:
            f_node(0, fi)
            fi += 1
        f_out(0)
        for i in range(N):
            f_node(1, i)
        f_out(1)

    nc.compile()
    return nc


_NC_CACHE = {}


def _get_program():
    if "nc" not in _NC_CACHE:
        _NC_CACHE["nc"] = _build_program()
    return _NC_CACHE["nc"]


def _bf16(x):
    import ml_dtypes

    return np.ascontiguousarray(np.asarray(x, np.float32).astype(
        ml_dtypes.bfloat16))


def _host_consts(W, embeddings, g_W1, g_b1, g_W2, g_b2, g_W3, g_b3,
                 f_W1, f_b1, f_W2, f_b2, f_W3, f_b3):
    f = np.float32
    W_adj = (W * (1.0 - np.eye(N, dtype=f))).astype(f)
    U = np.ascontiguousarray(g_W1[:D].T, dtype=f)                    # [A, N]
    C1 = np.ascontiguousarray((embeddings @ g_W1[D:] + g_b1).T, f)   # [A, N]
    s = W_adj.sum(axis=0)                                            # [N]
    C2 = (embeddings @ f_W1[D:] + f_b1 + np.outer(s, g_b3 @ f_W1[:D]))
    C2 = np.ascontiguousarray(C2.T, dtype=f)                         # [A, N]
    GW3P = np.zeros((A, NCH * A), f)
    FW1P = np.zeros((A, NCH * A), f)
    for c in range(NCH):
        GW3P[:, c * A + c * D : c * A + (c + 1) * D] = g_W3
        FW1P[c * D : (c + 1) * D, c * A : (c + 1) * A] = f_W1[:D]
    BD = np.kron(np.eye(NCH, dtype=f), W_adj).astype(f)
    VPX = np.zeros((A, N * VREG), f)
    for i in range(N):
        VPX[:, VREG * i + 4 * i + 3] = f_W3[:, i]
    return {
        "GW2": _bf16(g_W2),
        "FW2": _bf16(f_W2),
        "GW3P": _bf16(GW3P), "FW1P": _bf16(FW1P), "BD": _bf16(BD),
        "VPX": _bf16(VPX),
        "U": U, "UA": (ALPHA * U).astype(f), "C1": C1,
        "C1A": (ALPHA * C1).astype(f), "C2": C2,
        "GB2": np.ascontiguousarray(g_b2.reshape(A, 1), f),
        "FB2": np.ascontiguousarray(f_b2.reshape(A, 1), f),
    }


def _kernel_numpy(X, W, embeddings, g_W1, g_b1, g_W2, g_b2, g_W3, g_b3,
                  f_W1, f_b1, f_W2, f_b2, f_W3, f_b3, group_mask):
    # general fallback (non-identity group_mask)
    def lrelu(x):
        return np.where(x > 0, x, ALPHA * x)

    def mlp(x, W1, b1, W2, b2, W3, b3):
        h = lrelu(x @ W1 + b1)
        h = h + lrelu(h @ W2 + b2)
        return h @ W3 + b3

    n = W.shape[0]
    W_adj = W * (1.0 - np.eye(n, dtype=W.dtype))
    Xm = X[:, None, :] * group_mask
    E = np.broadcast_to(embeddings, (X.shape[0], n, embeddings.shape[1]))
    Xe = mlp(np.concatenate([Xm, E], 2), g_W1, g_b1, g_W2, g_b2, g_W3, g_b3)
    Xa = np.einsum("ji,bjd->bid", W_adj, Xe)
    Xr = mlp(np.concatenate([Xa, E], 2), f_W1, f_b1, f_W2, f_b2, f_W3, f_b3)
    return (Xr * group_mask).sum(axis=1).astype(np.float32)


def kernel(X, W, embeddings, g_W1, g_b1, g_W2, g_b2, g_W3, g_b3,
           f_W1, f_b1, f_W2, f_b2, f_W3, f_b3, group_mask, _run_kw=None):
    if not np.allclose(group_mask, np.eye(N, D, dtype=np.float32)):
        return _kernel_numpy(X, W, embeddings, g_W1, g_b1, g_W2, g_b2, g_W3,
                             g_b3, f_W1, f_b1, f_W2, f_b2, f_W3, f_b3,
                             group_mask)

    from concourse import bass_utils

    consts = _host_consts(W, embeddings, g_W1, g_b1, g_W2, g_b2, g_W3, g_b3,
                          f_W1, f_b1, f_W2, f_b2, f_W3, f_b3)
    XTB = _bf16(np.asarray(X, np.float32).T)  # [N, B]
    in_maps = []
    for k in range(NCORES):
        m = dict(consts)
        m["XTB"] = np.ascontiguousarray(XTB[:, k * BC : (k + 1) * BC])
        in_maps.append(m)

    nc = _get_program()
    res = bass_utils.run_bass_kernel_spmd(
        nc, in_maps, core_ids=list(range(NCORES)), **(_run_kw or {})
    )
    out = np.empty((B, D), np.float32)
    for k in range(NCORES):
        out[k * BC : (k + 1) * BC, :] = res.results[k]["OUT"].T
    out += f_b3.reshape(1, D).astype(np.float32)
    if _run_kw:
        kernel.last_results = res
    return out


# revision 94
# speedup vs baseline: 1.5079x; 1.0490x over previous
"""Trainium2 Bass kernel for ContractiveInvertibleGNN feed-forward.

Math (reference, with group_mask == I_32):
  out[b,i] = f_i( sum_j W_adj[j,i] * g_j(X[b,j]) )
where g_j: R -> R^32 and f_i: R^32 -> R are slices of two shared MLPs
(64->128->128->32 with a residual middle block, LeakyReLU 0.01):
  g: H1 = lrelu(X[b,j]*U_j + C1_j); H2 = H1 + lrelu(H1@W2g + b2g)
     X_emb = H2 @ W3g            (g_b3 folded into C2)
  f: Hf1 = lrelu(X_aggr@Wf1x + C2_i)
     Hf2 = Hf1 + lrelu(Hf1@Wf2 + bf2); out_i = Hf2 . V_i (+ f_b3_i on host)

Sharding: pure data-parallel over batch across 8 cores (2048 rows each).

Per-core schedule: batch is processed in 2 halves of 1024, emitted as a
software pipeline (per-engine execution follows emission order):
g(h0); then g(h1) interleaved with the agg of h0 (T1 transpose ->
block-diag matmul -> T2 transpose) and the first f(h0) nodes; then the
rest of f(h0) interleaved with agg(h1); then f(h1). gw3 / V-dot matmuls
are emitted with a 1-2 node skew so the PE never waits on the current
node's activations.

Most SBUF interchange tiles are bf16 (DVE 2x/4x modes, 1 cyc/row PE,
half DMA); PSUM stays f32. HW constraints honored: GPSIMD never touches
PSUM; StreamTranspose src/dst dtypes match (so Xa stays f32 and is
re-rounded to f32r for the PE by small SBUF->SBUF DMA hops); no mixed
32/16-bit matmul inputs. Engine assignment: Act = psum-sourced lrelus
(t2/hf1/tf) + a per-phase share of h1; DVE = most of the h1 lrelu
(tensor_scalar 4x path), transposes, psum->bf16 copies, tf tail, and a
slice of the f-residual add; Pool = SBUF-only work (h1 tail, most of
the f-residual add); PE = matmuls with the g-residual folded as double
accumulating matmuls. Final dot V_i routes node i / quarter c to PSUM
partition 4i+c via a padded stationary table (VPX), giving one
[128, 256] output copy + one DMA per half.
"""

import os
import sys

import numpy as np

for _p in ("/opt/trn_rl_repo", "/root/.axon_site/_ro/trn_rl_repo"):
    if os.path.isdir(_p) and _p not in sys.path:
        sys.path.insert(0, _p)

N = 32          # nodes
D = 32          # processed dim (== N, group_mask = I)
A = 128         # hidden width
B = 16384       # batch
NCORES = 8
BC = B // NCORES        # 2048 rows per core
NH = 2                  # batch halves per core
BH = BC // NH           # 1024 rows per half
NCH = 4                 # partition-group (quarter) stacking factor
CH = BH // NCH          # 256 cols per quarter
ALPHA = 0.01

# knobs
TG_POOL = 0             # t2 cols per half on Pool engine (rest on Act)
PF_DVE = 448            # tf cols per half on DVE (rest on Act)
VREG = A + 3            # per-node stationary region width in VPX


def _build_program():
    from contextlib import ExitStack

    from concourse import bacc, bass, mybir, tile

    f32 = mybir.dt.float32
    f32r = mybir.dt.float32r
    bf16 = mybir.dt.bfloat16
    LRELU = mybir.ActivationFunctionType.Lrelu
    ALU_MULT = mybir.AluOpType.mult
    ALU_ADD = mybir.AluOpType.add
    ALU_MAX = mybir.AluOpType.max

    nc = bacc.Bacc("TRN2", target_bir_lowering=False, debug=False)

    def din(name, shape, dt):
        return nc.dram_tensor(
            name, list(shape), dt, kind="ExternalInput"
        ).ap()

    xtb_d = din("XTB", (N, BC), bf16)
    gw2_d = din("GW2", (A, A), bf16)
    fw2_d = din("FW2", (A, A), bf16)
    gw3p_d = din("GW3P", (A, NCH * A), bf16)   # col-block c: g_W3 at cols 32c..
    fw1p_d = din("FW1P", (A, NCH * A), bf16)   # row-block c: f_W1[:32] rows 32c
    bd_d = din("BD", (A, A), bf16)             # kron(I4, W_adj)
    vpx_d = din("VPX", (A, N * VREG), bf16)    # region i: V_i at col 131i+4i+3
    u_d = din("U", (A, N), f32)
    ua_d = din("UA", (A, N), f32)
    c1_d = din("C1", (A, N), f32)
    c1a_d = din("C1A", (A, N), f32)
    c2_d = din("C2", (A, N), f32)
    gb2_d = din("GB2", (A, 1), f32)
    fb2_d = din("FB2", (A, 1), f32)
    out_d = nc.dram_tensor("OUT", [N, BC], f32, kind="ExternalOutput").ap()

    with tile.TileContext(nc) as tc, ExitStack() as ctx:
        const = ctx.enter_context(tc.tile_pool(name="const", bufs=1))
        bigp = ctx.enter_context(tc.tile_pool(name="big", bufs=2))
        workp = ctx.enter_context(tc.tile_pool(name="work", bufs=5))
        xbcp = ctx.enter_context(tc.tile_pool(name="xbc", bufs=5))
        lrp = ctx.enter_context(tc.tile_pool(name="lrp", bufs=2))
        outp = ctx.enter_context(tc.tile_pool(name="outs", bufs=2))
        xarp = ctx.enter_context(tc.tile_pool(name="xar", bufs=3))
        # PSUM bank budget (8 banks of 2KB): ppG 3 + ppF 4 + ppV 1.
        # ppG serves gw2-out, gw3-out pairs and agg windows; ppF serves
        # the f phase only, so g(h+1) never waits on f(h) ring slots.
        ppG = ctx.enter_context(tc.tile_pool(name="ppG", bufs=3, space="PSUM"))
        ppF = ctx.enter_context(tc.tile_pool(name="ppF", bufs=4, space="PSUM"))
        ppV = ctx.enter_context(tc.tile_pool(name="ppV", bufs=1, space="PSUM"))

        def load_const(ap_dram, shape):
            t = const.tile(list(shape), ap_dram.dtype,
                           tag=f"c_{ap_dram.tensor.name}")
            nc.sync.dma_start(t[:, :], ap_dram)
            return t

        st = [dict() for _ in range(NH)]
        u_s = load_const(u_d, (A, N))
        ua_s = load_const(ua_d, (A, N))
        c1_s = load_const(c1_d, (A, N))
        c1a_s = load_const(c1a_d, (A, N))
        gw2_s = load_const(gw2_d, (A, A))
        gb2_s = load_const(gb2_d, (A, 1))
        heavy = {}
        heavy["gw3p"] = load_const(gw3p_d, (A, NCH * A))
        heavy["bd"] = load_const(bd_d, (A, A))
        heavy["fw1p"] = load_const(fw1p_d, (A, NCH * A))
        heavy["fw2"] = load_const(fw2_d, (A, A))
        heavy["c2"] = load_const(c2_d, (A, N))
        heavy["fb2"] = load_const(fb2_d, (A, 1))

        # Per-half state for the software-pipelined emission below.
        TSP = BH - TG_POOL
        SPL = BH - PF_DVE
        TS = 8                  # T1 transpose steps per half
        WT = 512 // D           # 16 t per agg window
        NW = CH * D // 512      # 16 agg windows per half
        def g_node(h, j, h1_act=False, gpool=None):
            """Emit g work for node j; gw3+copy are skewed (emitted for
            j-1) so the PE never stalls on the current node's t2. The h1
            lrelu runs on Act or DVE per the h1_act flag, chosen by the
            emission schedule to balance per-phase engine load."""
            s = st[h]
            if j == 0:
                s["xe"] = bigp.tile([A, N * CH], bf16, tag="xe", name="xe")
                s["gq"] = {}
            if j in s.get("pre", {}):
                xbc = s["pre"].pop(j)
            else:
                xbc = xbcp.tile([A, BH], bf16, tag="xbc")
                nc.sync.dma_start(
                    xbc[:, :],
                    xtb_d[j : j + 1,
                          h * BH : (h + 1) * BH].partition_broadcast(A),
                )
            h1 = workp.tile([A, BH], bf16, tag="h1")
            uj = u_s[:, j : j + 1]
            cj = c1_s[:, j : j + 1]
            uaj = ua_s[:, j : j + 1]
            caj = c1a_s[:, j : j + 1]
            if h1_act:
                nc.scalar.activation(h1[:, :], xbc[:, :], LRELU,
                                     bias=cj, scale=uj, alpha=ALPHA)
            else:
                GP = 384  # h1 tail cols on the otherwise-idle Pool engine
                zt = lrp.tile([A, BH], bf16, tag="z")
                mt = lrp.tile([A, BH], bf16, tag="m")
                nc.vector.tensor_scalar(zt[:, : BH - GP], xbc[:, : BH - GP],
                                        uj, cj, ALU_MULT, ALU_ADD)
                nc.vector.tensor_scalar(mt[:, : BH - GP], xbc[:, : BH - GP],
                                        uaj, caj, ALU_MULT, ALU_ADD)
                nc.vector.tensor_tensor(h1[:, : BH - GP], zt[:, : BH - GP],
                                        mt[:, : BH - GP], ALU_MAX)
                nc.gpsimd.tensor_scalar(zt[:, BH - GP :], xbc[:, BH - GP :],
                                        uj, cj, ALU_MULT, ALU_ADD)
                nc.gpsimd.tensor_scalar(mt[:, BH - GP :], xbc[:, BH - GP :],
                                        uaj, caj, ALU_MULT, ALU_ADD)
                nc.vector.tensor_tensor(h1[:, BH - GP :], zt[:, BH - GP :],
                                        mt[:, BH - GP :], ALU_MAX)
            # middle residual block: t2 = lrelu(h1 @ gw2 + gb2)
            t2 = workp.tile([A, BH], bf16, tag="t2")
            for q in range(2):
                pa = (gpool or ppG).tile(
                    [A, 512], f32,
                    tag="pF" if gpool is ppF else "pG", name="pa")
                sl = slice(q * 512, (q + 1) * 512)
                nc.tensor.matmul(pa[:, :], gw2_s[:, :], h1[:, sl],
                                 start=True, stop=True)
                if q == 0:
                    nc.scalar.activation(t2[:, sl], pa[:, :], LRELU,
                                         bias=gb2_s[:, 0:1], alpha=ALPHA)
                else:
                    nc.scalar.activation(
                        t2[:, 512 : TSP], pa[:, : TSP - 512], LRELU,
                        bias=gb2_s[:, 0:1], alpha=ALPHA,
                    )
                    if TG_POOL:
                        gz = lrp.tile([A, TG_POOL], bf16, tag="gz")
                        nc.gpsimd.tensor_scalar(gz[:, :], pa[:, TSP - 512 :],
                                                ALPHA, None, ALU_MULT)
                        nc.gpsimd.tensor_tensor(t2[:, TSP:],
                                                pa[:, TSP - 512 :],
                                                gz[:, :], ALU_MAX)
            s["gq"][j] = (h1, t2)
            # skew: emit X_emb projection for the pair (j-3, j-2) so the
            # PE never stalls on the current node's t2.
            if j >= 3 and j % 2 == 1:
                g_emb(h, j - 3)

        def g_emb(h, j):
            """X_emb for nodes j and j+1 into one PSUM tile: gw3^T @
            (h1 + t2) with the residual folded as double accumulating
            matmuls; quarter c routed to psum rows 32c."""
            s = st[h]
            pm3_full = ppG.tile([A, 512], f32, tag="pG", name="pm3")
            for u in range(2):
                h1, t2 = s["gq"].pop(j + u)
                pm3 = pm3_full[:, u * CH : (u + 1) * CH]
                for c in range(NCH):
                    lt = heavy["gw3p"][:, c * A : (c + 1) * A]
                    csl = slice(c * CH, (c + 1) * CH)
                    nc.tensor.matmul(pm3[:, :], lt, h1[:, csl],
                                     start=(c == 0), stop=False)
                    nc.tensor.matmul(pm3[:, :], lt, t2[:, csl],
                                     start=False, stop=(c == NCH - 1))
            nc.vector.tensor_copy(s["xe"][:, j * CH : (j + 2) * CH],
                                  pm3_full[:, :])

        def agg_step(h, k):
            """Step k of the aggregation pipeline: 0..TS-1 are T1 block
            transposes, TS..TS+NW-1 are (block-diag matmul, T2) pairs."""
            s = st[h]
            if k == 0:
                s["xtile"] = bigp.tile([A, CH * D], bf16, tag="xt",
                                       name="xtile", bufs=1)
                s["xa"] = bigp.tile([A, N * CH], f32, tag="xa", name="xa")
            xe, xtile, xa = s["xe"], s["xtile"], s["xa"]
            if k < TS:
                xt3 = xe.rearrange("p (j t) -> p j t",
                                   j=N).transpose([0, 2, 1])
                xto = xtile.rearrange("p (t d) -> p t d", d=D)
                tstep = CH // TS
                nc.vector.transpose(
                    xto[:, k * tstep : (k + 1) * tstep, :],
                    xt3[:, k * tstep : (k + 1) * tstep, :],
                )
                return
            w = k - TS
            xa3 = xa.rearrange("p (i t) -> p i t", i=N).transpose([0, 2, 1])
            pg = ppG.tile([A, 512], f32, tag="pG", name="pg")
            nc.tensor.matmul(
                pg[:, :], heavy["bd"][:, :],
                xtile[:, w * 512 : (w + 1) * 512], start=True, stop=True,
            )
            nc.vector.transpose(
                xa3[:, w * WT : (w + 1) * WT, :],
                pg.rearrange("p (t d) -> p t d", d=D)[:, :, :],
            )

        def f_hop(h, i):
            # rounded-bits SBUF->SBUF DMA: xa (f32) -> f32r for the PE,
            # two nodes per transfer
            s = st[h]
            xr = xarp.tile([A, 2 * CH], f32r, tag="xr", name="xr")
            nc.sync.dma_start(
                xr[:, :], s["xa"].bitcast(f32r)[:, i * CH : (i + 2) * CH])
            s["hop"][i] = xr

        def f_node(h, i):
            """Emit f work for node i; the V-dot is skewed (emitted for
            i-1) so the PE never stalls on the current node's hf2."""
            s = st[h]
            if i == 0:
                s["vps"] = ppV.tile([A, CH], f32, tag="pV", name="vps")
                s["fq"] = {}
                s["hop"] = {}
                f_hop(h, 0)
                f_hop(h, 2)
                f_hop(h, 4)
            if i % 2 == 0 and i + 6 < N:
                f_hop(h, i + 6)
            xr = s["hop"].pop(i - 1) if i % 2 else s["hop"][i]
            rhs = xr[:, (i % 2) * CH : (i % 2 + 1) * CH]
            hf1 = workp.tile([A, BH], bf16, tag="hf1", bufs=3)
            for q in range(2):
                paf = ppF.tile([A, 512], f32, tag="pF", name="paf")
                for cc in range(2):
                    c = 2 * q + cc
                    nc.tensor.matmul(
                        paf[:, cc * CH : (cc + 1) * CH],
                        heavy["fw1p"][:, c * A : (c + 1) * A], rhs,
                        start=True, stop=True,
                    )
                sl = slice(q * 512, (q + 1) * 512)
                nc.scalar.activation(hf1[:, sl], paf[:, :], LRELU,
                                     bias=heavy["c2"][:, i : i + 1],
                                     alpha=ALPHA)
                # skew: V-dots from three nodes back fill the PE while
                # this node's activations run.
                if q == 0 and i > 2:
                    f_vdot(h, i - 3)
            tf = workp.tile([A, BH], bf16, tag="tf", bufs=3)
            for q in range(2):
                pbf = ppF.tile([A, 512], f32, tag="pF", name="pbf")
                sl = slice(q * 512, (q + 1) * 512)
                for cc in range(2):
                    c = 2 * q + cc
                    csl = slice(c * CH, (c + 1) * CH)
                    nc.tensor.matmul(pbf[:, cc * CH : (cc + 1) * CH],
                                     heavy["fw2"][:, :], hf1[:, csl],
                                     start=True, stop=True)
                if q == 0:
                    nc.scalar.activation(tf[:, sl], pbf[:, :], LRELU,
                                         bias=heavy["fb2"][:, 0:1],
                                         alpha=ALPHA)
                else:
                    asl = 512 - (PF_DVE if h == 0 else 512)
                    if asl:
                        nc.scalar.activation(tf[:, 512 : 512 + asl],
                                             pbf[:, :asl], LRELU,
                                             bias=heavy["fb2"][:, 0:1],
                                             alpha=ALPHA)
                    dz = lrp.tile([A, 512], bf16, tag="dz")
                    dz = dz[:, : 512 - asl]
                    nc.vector.tensor_scalar(dz[:, :], pbf[:, asl:],
                                            ALPHA, None, ALU_MULT)
                    nc.vector.tensor_tensor(tf[:, 512 + asl :],
                                            pbf[:, asl:], dz[:, :],
                                            ALU_MAX)
            hf2 = workp.tile([A, BH], bf16, tag="hf2", bufs=5)
            # residual add split across the SBUF-only Pool engine and DVE
            nc.gpsimd.tensor_tensor(hf2[:, :640], hf1[:, :640],
                                    tf[:, :640], ALU_ADD)
            nc.vector.tensor_tensor(hf2[:, 640:], hf1[:, 640:],
                                    tf[:, 640:], ALU_ADD)
            s["fq"][i] = hf2

        def f_vdot(h, i):
            # out_i = hf2 . V_i via accumulating matmuls; stationary
            # window puts V_i at psum partition 4i+c.
            s = st[h]
            hf2 = s["fq"].pop(i)
            for c in range(NCH):
                base = VREG * i + 3 - c
                lt = heavy["vpx"][:, base : base + A]
                sl = slice(c * CH, (c + 1) * CH)
                nc.tensor.matmul(
                    s["vps"][:, :], lt, hf2[:, sl],
                    start=(i == 0 and c == 0),
                    stop=(i == N - 1 and c == NCH - 1),
                )

        def f_out(h):
            f_vdot(h, N - 3)
            f_vdot(h, N - 2)
            f_vdot(h, N - 1)
            osb = outp.tile([A, CH], f32, tag="o")
            nc.vector.tensor_copy(osb[:, :], st[h]["vps"][:, :])
            nc.gpsimd.dma_start(
                out_d[:, h * BH : (h + 1) * BH].rearrange(
                    "i (c t) -> i c t", c=NCH),
                osb[:, :],
            )

        # ---- software-pipelined emission across the two halves ----
        NAGG = TS + NW  # 20 agg steps per half
        for j in range(N):
            g_node(0, j, h1_act=(j % 8 == 7), gpool=ppF)
            if j == 0:
                heavy["vpx"] = load_const(vpx_d, (A, N * VREG))
        g_emb(0, N - 2)
        # agg(0) interleaved with the first 20 nodes of g(1); the last 12
        # nodes of g(1) interleave 1:1 with the first 12 nodes of f(0).
        fi = 0
        for k in range(N):
            g_node(1, k, h1_act=(k < NAGG and k % 3 == 0),
                   gpool=(ppF if k < NAGG - 2 else None))
            if k < NAGG:
                agg_step(0, k)
            else:
                f_node(0, fi)
                fi += 1
        g_emb(1, N - 2)
        # remaining f(0) nodes interleave with agg(1)
        for k in range(NAGG):
            f_node(0, fi)
            fi += 1
            agg_step(1, k)
        while fi < N:
            f_node(0, fi)
            fi += 1
        f_out(0)
        for i in range(N):
            f_node(1, i)
        f_out(1)

    nc.compile()
    return nc


_NC_CACHE = {}


def _get_program():
    if "nc" not in _NC_CACHE:
        _NC_CACHE["nc"] = _build_program()
    return _NC_CACHE["nc"]


def _bf16(x):
    import ml_dtypes

    return np.ascontiguousarray(np.asarray(x, np.float32).astype(
        ml_dtypes.bfloat16))


def _host_consts(W, embeddings, g_W1, g_b1, g_W2, g_b2, g_W3, g_b3,
                 f_W1, f_b1, f_W2, f_b2, f_W3, f_b3):
    f = np.float32
    W_adj = (W * (1.0 - np.eye(N, dtype=f))).astype(f)
    U = np.ascontiguousarray(g_W1[:D].T, dtype=f)                    # [A, N]
    C1 = np.ascontiguousarray((embeddings @ g_W1[D:] + g_b1).T, f)   # [A, N]
    s = W_adj.sum(axis=0)                                            # [N]
    C2 = (embeddings @ f_W1[D:] + f_b1 + np.outer(s, g_b3 @ f_W1[:D]))
    C2 = np.ascontiguousarray(C2.T, dtype=f)                         # [A, N]
    GW3P = np.zeros((A, NCH * A), f)
    FW1P = np.zeros((A, NCH * A), f)
    for c in range(NCH):
        GW3P[:, c * A + c * D : c * A + (c + 1) * D] = g_W3
        FW1P[c * D : (c + 1) * D, c * A : (c + 1) * A] = f_W1[:D]
    BD = np.kron(np.eye(NCH, dtype=f), W_adj).astype(f)
    VPX = np.zeros((A, N * VREG), f)
    for i in range(N):
        VPX[:, VREG * i + 4 * i + 3] = f_W3[:, i]
    return {
        "GW2": _bf16(g_W2),
        "FW2": _bf16(f_W2),
        "GW3P": _bf16(GW3P), "FW1P": _bf16(FW1P), "BD": _bf16(BD),
        "VPX": _bf16(VPX),
        "U": U, "UA": (ALPHA * U).astype(f), "C1": C1,
        "C1A": (ALPHA * C1).astype(f), "C2": C2,
        "GB2": np.ascontiguousarray(g_b2.reshape(A, 1), f),
        "FB2": np.ascontiguousarray(f_b2.reshape(A, 1), f),
    }


def _kernel_numpy(X, W, embeddings, g_W1, g_b1, g_W2, g_b2, g_W3, g_b3,
                  f_W1, f_b1, f_W2, f_b2, f_W3, f_b3, group_mask):
    # general fallback (non-identity group_mask)
    def lrelu(x):
        return np.where(x > 0, x, ALPHA * x)

    def mlp(x, W1, b1, W2, b2, W3, b3):
        h = lrelu(x @ W1 + b1)
        h = h + lrelu(h @ W2 + b2)
        return h @ W3 + b3

    n = W.shape[0]
    W_adj = W * (1.0 - np.eye(n, dtype=W.dtype))
    Xm = X[:, None, :] * group_mask
    E = np.broadcast_to(embeddings, (X.shape[0], n, embeddings.shape[1]))
    Xe = mlp(np.concatenate([Xm, E], 2), g_W1, g_b1, g_W2, g_b2, g_W3, g_b3)
    Xa = np.einsum("ji,bjd->bid", W_adj, Xe)
    Xr = mlp(np.concatenate([Xa, E], 2), f_W1, f_b1, f_W2, f_b2, f_W3, f_b3)
    return (Xr * group_mask).sum(axis=1).astype(np.float32)


def kernel(X, W, embeddings, g_W1, g_b1, g_W2, g_b2, g_W3, g_b3,
           f_W1, f_b1, f_W2, f_b2, f_W3, f_b3, group_mask, _run_kw=None):
    if not np.allclose(group_mask, np.eye(N, D, dtype=np.float32)):
        return _kernel_numpy(X, W, embeddings, g_W1, g_b1, g_W2, g_b2, g_W3,
                             g_b3, f_W1, f_b1, f_W2, f_b2, f_W3, f_b3,
                             group_mask)

    from concourse import bass_utils

    consts = _host_consts(W, embeddings, g_W1, g_b1, g_W2, g_b2, g_W3, g_b3,
                          f_W1, f_b1, f_W2, f_b2, f_W3, f_b3)
    XTB = _bf16(np.asarray(X, np.float32).T)  # [N, B]
    in_maps = []
    for k in range(NCORES):
        m = dict(consts)
        m["XTB"] = np.ascontiguousarray(XTB[:, k * BC : (k + 1) * BC])
        in_maps.append(m)

    nc = _get_program()
    res = bass_utils.run_bass_kernel_spmd(
        nc, in_maps, core_ids=list(range(NCORES)), **(_run_kw or {})
    )
    out = np.empty((B, D), np.float32)
    for k in range(NCORES):
        out[k * BC : (k + 1) * BC, :] = res.results[k]["OUT"].T
    out += f_b3.reshape(1, D).astype(np.float32)
    if _run_kw:
        kernel.last_results = res
    return out


# revision 101
# speedup vs baseline: 1.5160x; 1.0053x over previous
"""Trainium2 Bass kernel for ContractiveInvertibleGNN feed-forward.

Math (reference, with group_mask == I_32):
  out[b,i] = f_i( sum_j W_adj[j,i] * g_j(X[b,j]) )
where g_j: R -> R^32 and f_i: R^32 -> R are slices of two shared MLPs
(64->128->128->32 with a residual middle block, LeakyReLU 0.01):
  g: H1 = lrelu(X[b,j]*U_j + C1_j); H2 = H1 + lrelu(H1@W2g + b2g)
     X_emb = H2 @ W3g            (g_b3 folded into C2)
  f: Hf1 = lrelu(X_aggr@Wf1x + C2_i)
     Hf2 = Hf1 + lrelu(Hf1@Wf2 + bf2); out_i = Hf2 . V_i (+ f_b3_i on host)

Sharding: pure data-parallel over batch across 8 cores (2048 rows each).

Per-core schedule: batch is processed in 2 halves of 1024, emitted as a
software pipeline (per-engine execution follows emission order):
g(h0); then g(h1) interleaved with the agg of h0 (T1 transpose ->
block-diag matmul -> T2 transpose) and the first f(h0) nodes; then the
rest of f(h0) interleaved with agg(h1); then f(h1). gw3 / V-dot matmuls
are emitted with a 1-2 node skew so the PE never waits on the current
node's activations.

Most SBUF interchange tiles are bf16 (DVE 2x/4x modes, 1 cyc/row PE,
half DMA); PSUM stays f32. HW constraints honored: GPSIMD never touches
PSUM; StreamTranspose src/dst dtypes match (so Xa stays f32 and is
re-rounded to f32r for the PE by small SBUF->SBUF DMA hops); no mixed
32/16-bit matmul inputs. Engine assignment: Act = psum-sourced lrelus
(t2/hf1/tf) + a per-phase share of h1; DVE = most of the h1 lrelu
(tensor_scalar 4x path), transposes, psum->bf16 copies, tf tail, and a
slice of the f-residual add; Pool = SBUF-only work (h1 tail, most of
the f-residual add); PE = matmuls with the g-residual folded as double
accumulating matmuls. Final dot V_i routes node i / quarter c to PSUM
partition 4i+c via a padded stationary table (VPX), giving one
[128, 256] output copy + one DMA per half.
"""

import os
import sys

import numpy as np

for _p in ("/opt/trn_rl_repo", "/root/.axon_site/_ro/trn_rl_repo"):
    if os.path.isdir(_p) and _p not in sys.path:
        sys.path.insert(0, _p)

N = 32          # nodes
D = 32          # processed dim (== N, group_mask = I)
A = 128         # hidden width
B = 16384       # batch
NCORES = 8
BC = B // NCORES        # 2048 rows per core
NH = 2                  # batch halves per core
BH = BC // NH           # 1024 rows per half
NCH = 4                 # partition-group (quarter) stacking factor
CH = BH // NCH          # 256 cols per quarter
ALPHA = 0.01

# knobs
TG_POOL = 0             # t2 cols per half on Pool engine (rest on Act)
PF_DVE = 448            # tf cols per half on DVE (rest on Act)
VREG = A + 3            # per-node stationary region width in VPX


def _build_program():
    from contextlib import ExitStack

    from concourse import bacc, bass, mybir, tile

    f32 = mybir.dt.float32
    f32r = mybir.dt.float32r
    bf16 = mybir.dt.bfloat16
    LRELU = mybir.ActivationFunctionType.Lrelu
    ALU_MULT = mybir.AluOpType.mult
    ALU_ADD = mybir.AluOpType.add
    ALU_MAX = mybir.AluOpType.max

    nc = bacc.Bacc("TRN2", target_bir_lowering=False, debug=False)

    def din(name, shape, dt):
        return nc.dram_tensor(
            name, list(shape), dt, kind="ExternalInput"
        ).ap()

    xtb_d = din("XTB", (N, BC), bf16)
    gw2_d = din("GW2", (A, A), bf16)
    fw2_d = din("FW2", (A, A), bf16)
    gw3p_d = din("GW3P", (A, NCH * A), bf16)   # col-block c: g_W3 at cols 32c..
    fw1p_d = din("FW1P", (A, NCH * A), bf16)   # row-block c: f_W1[:32] rows 32c
    bd_d = din("BD", (A, A), bf16)             # kron(I4, W_adj)
    vpx_d = din("VPX", (A, N * VREG), bf16)    # region i: V_i at col 131i+4i+3
    u_d = din("U", (A, N), f32)
    ua_d = din("UA", (A, N), f32)
    c1_d = din("C1", (A, N), f32)
    c1a_d = din("C1A", (A, N), f32)
    c2_d = din("C2", (A, N), f32)
    gb2_d = din("GB2", (A, 1), f32)
    fb2_d = din("FB2", (A, 1), f32)
    out_d = nc.dram_tensor("OUT", [N, BC], f32, kind="ExternalOutput").ap()

    with tile.TileContext(nc) as tc, ExitStack() as ctx:
        const = ctx.enter_context(tc.tile_pool(name="const", bufs=1))
        bigp = ctx.enter_context(tc.tile_pool(name="big", bufs=2))
        workp = ctx.enter_context(tc.tile_pool(name="work", bufs=5))
        xbcp = ctx.enter_context(tc.tile_pool(name="xbc", bufs=5))
        lrp = ctx.enter_context(tc.tile_pool(name="lrp", bufs=2))
        outp = ctx.enter_context(tc.tile_pool(name="outs", bufs=2))
        xarp = ctx.enter_context(tc.tile_pool(name="xar", bufs=3))
        # PSUM bank budget (8 banks of 2KB): ppG 3 + ppF 4 + ppV 1.
        # ppG serves gw2-out, gw3-out pairs and agg windows; ppF serves
        # the f phase only, so g(h+1) never waits on f(h) ring slots.
        ppG = ctx.enter_context(tc.tile_pool(name="ppG", bufs=3, space="PSUM"))
        ppF = ctx.enter_context(tc.tile_pool(name="ppF", bufs=4, space="PSUM"))
        ppV = ctx.enter_context(tc.tile_pool(name="ppV", bufs=1, space="PSUM"))

        def load_const(ap_dram, shape):
            t = const.tile(list(shape), ap_dram.dtype,
                           tag=f"c_{ap_dram.tensor.name}")
            nc.sync.dma_start(t[:, :], ap_dram)
            return t

        st = [dict() for _ in range(NH)]
        u_s = load_const(u_d, (A, N))
        ua_s = load_const(ua_d, (A, N))
        c1_s = load_const(c1_d, (A, N))
        c1a_s = load_const(c1a_d, (A, N))
        gw2_s = load_const(gw2_d, (A, A))
        gb2_s = load_const(gb2_d, (A, 1))
        # prefetch the first broadcast rows ahead of the heavy tables so
        # compute starts immediately; heavy consts are first needed at
        # the j=3 X_emb projection.
        st[0]["pre"] = {}
        for _pj in range(4):
            _xpre = xbcp.tile([A, BH], bf16, tag="xbc", name="xbc_pre")
            nc.sync.dma_start(
                _xpre[:, :],
                xtb_d[_pj : _pj + 1, 0:BH].partition_broadcast(A),
            )
            st[0]["pre"][_pj] = _xpre
        heavy = {}
        heavy["gw3p"] = load_const(gw3p_d, (A, NCH * A))
        heavy["bd"] = load_const(bd_d, (A, A))
        heavy["fw1p"] = load_const(fw1p_d, (A, NCH * A))
        heavy["fw2"] = load_const(fw2_d, (A, A))
        heavy["c2"] = load_const(c2_d, (A, N))
        heavy["fb2"] = load_const(fb2_d, (A, 1))

        # Per-half state for the software-pipelined emission below.
        TSP = BH - TG_POOL
        SPL = BH - PF_DVE
        TS = 8                  # T1 transpose steps per half
        WT = 512 // D           # 16 t per agg window
        NW = CH * D // 512      # 16 agg windows per half
        def g_node(h, j, h1_act=False, gpool=None):
            """Emit g work for node j; gw3+copy are skewed (emitted for
            j-1) so the PE never stalls on the current node's t2. The h1
            lrelu runs on Act or DVE per the h1_act flag, chosen by the
            emission schedule to balance per-phase engine load."""
            s = st[h]
            if j == 0:
                s["xe"] = bigp.tile([A, N * CH], bf16, tag="xe", name="xe")
                s["gq"] = {}
            if j in s.get("pre", {}):
                xbc = s["pre"].pop(j)
            else:
                xbc = xbcp.tile([A, BH], bf16, tag="xbc")
                nc.sync.dma_start(
                    xbc[:, :],
                    xtb_d[j : j + 1,
                          h * BH : (h + 1) * BH].partition_broadcast(A),
                )
            h1 = workp.tile([A, BH], bf16, tag="h1")
            uj = u_s[:, j : j + 1]
            cj = c1_s[:, j : j + 1]
            uaj = ua_s[:, j : j + 1]
            caj = c1a_s[:, j : j + 1]
            if h1_act:
                nc.scalar.activation(h1[:, :], xbc[:, :], LRELU,
                                     bias=cj, scale=uj, alpha=ALPHA)
            else:
                GP = 384  # h1 tail cols on the otherwise-idle Pool engine
                zt = lrp.tile([A, BH], bf16, tag="z")
                mt = lrp.tile([A, BH], bf16, tag="m")
                nc.vector.tensor_scalar(zt[:, : BH - GP], xbc[:, : BH - GP],
                                        uj, cj, ALU_MULT, ALU_ADD)
                nc.vector.tensor_scalar(mt[:, : BH - GP], xbc[:, : BH - GP],
                                        uaj, caj, ALU_MULT, ALU_ADD)
                nc.vector.tensor_tensor(h1[:, : BH - GP], zt[:, : BH - GP],
                                        mt[:, : BH - GP], ALU_MAX)
                nc.gpsimd.tensor_scalar(zt[:, BH - GP :], xbc[:, BH - GP :],
                                        uj, cj, ALU_MULT, ALU_ADD)
                nc.gpsimd.tensor_scalar(mt[:, BH - GP :], xbc[:, BH - GP :],
                                        uaj, caj, ALU_MULT, ALU_ADD)
                nc.vector.tensor_tensor(h1[:, BH - GP :], zt[:, BH - GP :],
                                        mt[:, BH - GP :], ALU_MAX)
            # middle residual block: t2 = lrelu(h1 @ gw2 + gb2)
            t2 = workp.tile([A, BH], bf16, tag="t2")
            for q in range(2):
                pa = (gpool or ppG).tile(
                    [A, 512], f32,
                    tag="pF" if gpool is ppF else "pG", name="pa")
                sl = slice(q * 512, (q + 1) * 512)
                nc.tensor.matmul(pa[:, :], gw2_s[:, :], h1[:, sl],
                                 start=True, stop=True)
                if q == 0:
                    nc.scalar.activation(t2[:, sl], pa[:, :], LRELU,
                                         bias=gb2_s[:, 0:1], alpha=ALPHA)
                else:
                    nc.scalar.activation(
                        t2[:, 512 : TSP], pa[:, : TSP - 512], LRELU,
                        bias=gb2_s[:, 0:1], alpha=ALPHA,
                    )
                    if TG_POOL:
                        gz = lrp.tile([A, TG_POOL], bf16, tag="gz")
                        nc.gpsimd.tensor_scalar(gz[:, :], pa[:, TSP - 512 :],
                                                ALPHA, None, ALU_MULT)
                        nc.gpsimd.tensor_tensor(t2[:, TSP:],
                                                pa[:, TSP - 512 :],
                                                gz[:, :], ALU_MAX)
            s["gq"][j] = (h1, t2)
            # skew: emit X_emb projection for the pair (j-3, j-2) so the
            # PE never stalls on the current node's t2.
            if j >= 3 and j % 2 == 1:
                g_emb(h, j - 3)

        def g_emb(h, j):
            """X_emb for nodes j and j+1 into one PSUM tile: gw3^T @
            (h1 + t2) with the residual folded as double accumulating
            matmuls; quarter c routed to psum rows 32c."""
            s = st[h]
            pm3_full = ppG.tile([A, 512], f32, tag="pG", name="pm3")
            for u in range(2):
                h1, t2 = s["gq"].pop(j + u)
                pm3 = pm3_full[:, u * CH : (u + 1) * CH]
                for c in range(NCH):
                    lt = heavy["gw3p"][:, c * A : (c + 1) * A]
                    csl = slice(c * CH, (c + 1) * CH)
                    nc.tensor.matmul(pm3[:, :], lt, h1[:, csl],
                                     start=(c == 0), stop=False)
                    nc.tensor.matmul(pm3[:, :], lt, t2[:, csl],
                                     start=False, stop=(c == NCH - 1))
            nc.vector.tensor_copy(s["xe"][:, j * CH : (j + 2) * CH],
                                  pm3_full[:, :])

        def agg_step(h, k):
            """Step k of the aggregation pipeline: 0..TS-1 are T1 block
            transposes, TS..TS+NW-1 are (block-diag matmul, T2) pairs."""
            s = st[h]
            if k == 0:
                s["xtile"] = bigp.tile([A, CH * D], bf16, tag="xt",
                                       name="xtile", bufs=1)
                s["xa"] = bigp.tile([A, N * CH], f32, tag="xa", name="xa")
            xe, xtile, xa = s["xe"], s["xtile"], s["xa"]
            if k < TS:
                xt3 = xe.rearrange("p (j t) -> p j t",
                                   j=N).transpose([0, 2, 1])
                xto = xtile.rearrange("p (t d) -> p t d", d=D)
                tstep = CH // TS
                nc.vector.transpose(
                    xto[:, k * tstep : (k + 1) * tstep, :],
                    xt3[:, k * tstep : (k + 1) * tstep, :],
                )
                return
            w = k - TS
            xa3 = xa.rearrange("p (i t) -> p i t", i=N).transpose([0, 2, 1])
            pg = ppG.tile([A, 512], f32, tag="pG", name="pg")
            nc.tensor.matmul(
                pg[:, :], heavy["bd"][:, :],
                xtile[:, w * 512 : (w + 1) * 512], start=True, stop=True,
            )
            nc.vector.transpose(
                xa3[:, w * WT : (w + 1) * WT, :],
                pg.rearrange("p (t d) -> p t d", d=D)[:, :, :],
            )

        def f_hop(h, i):
            # rounded-bits SBUF->SBUF DMA: xa (f32) -> f32r for the PE,
            # two nodes per transfer
            s = st[h]
            xr = xarp.tile([A, 2 * CH], f32r, tag="xr", name="xr")
            nc.sync.dma_start(
                xr[:, :], s["xa"].bitcast(f32r)[:, i * CH : (i + 2) * CH])
            s["hop"][i] = xr

        def f_node(h, i):
            """Emit f work for node i; the V-dot is skewed (emitted for
            i-1) so the PE never stalls on the current node's hf2."""
            s = st[h]
            if i == 0:
                s["vps"] = ppV.tile([A, CH], f32, tag="pV", name="vps")
                s["fq"] = {}
                s["hop"] = {}
                f_hop(h, 0)
                f_hop(h, 2)
                f_hop(h, 4)
            if i % 2 == 0 and i + 6 < N:
                f_hop(h, i + 6)
            xr = s["hop"].pop(i - 1) if i % 2 else s["hop"][i]
            rhs = xr[:, (i % 2) * CH : (i % 2 + 1) * CH]
            hf1 = workp.tile([A, BH], bf16, tag="hf1", bufs=3)
            for q in range(2):
                paf = ppF.tile([A, 512], f32, tag="pF", name="paf")
                for cc in range(2):
                    c = 2 * q + cc
                    nc.tensor.matmul(
                        paf[:, cc * CH : (cc + 1) * CH],
                        heavy["fw1p"][:, c * A : (c + 1) * A], rhs,
                        start=True, stop=True,
                    )
                sl = slice(q * 512, (q + 1) * 512)
                nc.scalar.activation(hf1[:, sl], paf[:, :], LRELU,
                                     bias=heavy["c2"][:, i : i + 1],
                                     alpha=ALPHA)
                # skew: V-dots from three nodes back fill the PE while
                # this node's activations run.
                if q == 0 and i > 2:
                    f_vdot(h, i - 3)
            tf = workp.tile([A, BH], bf16, tag="tf", bufs=3)
            for q in range(2):
                pbf = ppF.tile([A, 512], f32, tag="pF", name="pbf")
                sl = slice(q * 512, (q + 1) * 512)
                for cc in range(2):
                    c = 2 * q + cc
                    csl = slice(c * CH, (c + 1) * CH)
                    nc.tensor.matmul(pbf[:, cc * CH : (cc + 1) * CH],
                                     heavy["fw2"][:, :], hf1[:, csl],
                                     start=True, stop=True)
                if q == 0:
                    nc.scalar.activation(tf[:, sl], pbf[:, :], LRELU,
                                         bias=heavy["fb2"][:, 0:1],
                                         alpha=ALPHA)
                else:
                    asl = 512 - (PF_DVE if h == 0 else 512)
                    if asl:
                        nc.scalar.activation(tf[:, 512 : 512 + asl],
                                             pbf[:, :asl], LRELU,
                                             bias=heavy["fb2"][:, 0:1],
                                             alpha=ALPHA)
                    dz = lrp.tile([A, 512], bf16, tag="dz")
                    dz = dz[:, : 512 - asl]
                    nc.vector.tensor_scalar(dz[:, :], pbf[:, asl:],
                                            ALPHA, None, ALU_MULT)
                    nc.vector.tensor_tensor(tf[:, 512 + asl :],
                                            pbf[:, asl:], dz[:, :],
                                            ALU_MAX)
            hf2 = workp.tile([A, BH], bf16, tag="hf2", bufs=5)
            # residual add split across the SBUF-only Pool engine and DVE
            nc.gpsimd.tensor_tensor(hf2[:, :640], hf1[:, :640],
                                    tf[:, :640], ALU_ADD)
            nc.vector.tensor_tensor(hf2[:, 640:], hf1[:, 640:],
                                    tf[:, 640:], ALU_ADD)
            s["fq"][i] = hf2

        def f_vdot(h, i):
            # out_i = hf2 . V_i via accumulating matmuls; stationary
            # window puts V_i at psum partition 4i+c.
            s = st[h]
            hf2 = s["fq"].pop(i)
            for c in range(NCH):
                base = VREG * i + 3 - c
                lt = heavy["vpx"][:, base : base + A]
                sl = slice(c * CH, (c + 1) * CH)
                nc.tensor.matmul(
                    s["vps"][:, :], lt, hf2[:, sl],
                    start=(i == 0 and c == 0),
                    stop=(i == N - 1 and c == NCH - 1),
                )

        def f_out(h):
            f_vdot(h, N - 3)
            f_vdot(h, N - 2)
            f_vdot(h, N - 1)
            osb = outp.tile([A, CH], f32, tag="o")
            nc.vector.tensor_copy(osb[:, :], st[h]["vps"][:, :])
            nc.gpsimd.dma_start(
                out_d[:, h * BH : (h + 1) * BH].rearrange(
                    "i (c t) -> i c t", c=NCH),
                osb[:, :],
            )

        # ---- software-pipelined emission across the two halves ----
        NAGG = TS + NW  # 20 agg steps per half
        for j in range(N):
            g_node(0, j, h1_act=(j % 8 == 7), gpool=ppF)
            if j == 0:
                heavy["vpx"] = load_const(vpx_d, (A, N * VREG))
        g_emb(0, N - 2)
        # agg(0) interleaved with the first 20 nodes of g(1); the last 12
        # nodes of g(1) interleave 1:1 with the first 12 nodes of f(0).
        fi = 0
        for k in range(N):
            g_node(1, k, h1_act=(k < NAGG and k % 3 == 0),
                   gpool=(ppF if k < NAGG - 2 else None))
            if k < NAGG:
                agg_step(0, k)
            else:
                f_node(0, fi)
                fi += 1
        g_emb(1, N - 2)
        # remaining f(0) nodes interleave with agg(1)
        for k in range(NAGG):
            if fi < N:
                f_node(0, fi)
                fi += 1
            agg_step(1, k)
        while fi < N:
            f_node(0, fi)
            fi += 1
        f_out(0)
        for i in range(N):
            f_node(1, i)
        f_out(1)

    nc.compile()
    return nc


_NC_CACHE = {}


def _get_program():
    if "nc" not in _NC_CACHE:
        _NC_CACHE["nc"] = _build_program()
    return _NC_CACHE["nc"]


def _bf16(x):
    import ml_dtypes

    return np.ascontiguousarray(np.asarray(x, np.float32).astype(
        ml_dtypes.bfloat16))


def _host_consts(W, embeddings, g_W1, g_b1, g_W2, g_b2, g_W3, g_b3,
                 f_W1, f_b1, f_W2, f_b2, f_W3, f_b3):
    f = np.float32
    W_adj = (W * (1.0 - np.eye(N, dtype=f))).astype(f)
    U = np.ascontiguousarray(g_W1[:D].T, dtype=f)                    # [A, N]
    C1 = np.ascontiguousarray((embeddings @ g_W1[D:] + g_b1).T, f)   # [A, N]
    s = W_adj.sum(axis=0)                                            # [N]
    C2 = (embeddings @ f_W1[D:] + f_b1 + np.outer(s, g_b3 @ f_W1[:D]))
    C2 = np.ascontiguousarray(C2.T, dtype=f)                         # [A, N]
    GW3P = np.zeros((A, NCH * A), f)
    FW1P = np.zeros((A, NCH * A), f)
    for c in range(NCH):
        GW3P[:, c * A + c * D : c * A + (c + 1) * D] = g_W3
        FW1P[c * D : (c + 1) * D, c * A : (c + 1) * A] = f_W1[:D]
    BD = np.kron(np.eye(NCH, dtype=f), W_adj).astype(f)
    VPX = np.zeros((A, N * VREG), f)
    for i in range(N):
        VPX[:, VREG * i + 4 * i + 3] = f_W3[:, i]
    return {
        "GW2": _bf16(g_W2),
        "FW2": _bf16(f_W2),
        "GW3P": _bf16(GW3P), "FW1P": _bf16(FW1P), "BD": _bf16(BD),
        "VPX": _bf16(VPX),
        "U": U, "UA": (ALPHA * U).astype(f), "C1": C1,
        "C1A": (ALPHA * C1).astype(f), "C2": C2,
        "GB2": np.ascontiguousarray(g_b2.reshape(A, 1), f),
        "FB2": np.ascontiguousarray(f_b2.reshape(A, 1), f),
    }


def _kernel_numpy(X, W, embeddings, g_W1, g_b1, g_W2, g_b2, g_W3, g_b3,
                  f_W1, f_b1, f_W2, f_b2, f_W3, f_b3, group_mask):
    # general fallback (non-identity group_mask)
    def lrelu(x):
        return np.where(x > 0, x, ALPHA * x)

    def mlp(x, W1, b1, W2, b2, W3, b3):
        h = lrelu(x @ W1 + b1)
        h = h + lrelu(h @ W2 + b2)
        return h @ W3 + b3

    n = W.shape[0]
    W_adj = W * (1.0 - np.eye(n, dtype=W.dtype))
    Xm = X[:, None, :] * group_mask
    E = np.broadcast_to(embeddings, (X.shape[0], n, embeddings.shape[1]))
    Xe = mlp(np.concatenate([Xm, E], 2), g_W1, g_b1, g_W2, g_b2, g_W3, g_b3)
    Xa = np.einsum("ji,bjd->bid", W_adj, Xe)
    Xr = mlp(np.concatenate([Xa, E], 2), f_W1, f_b1, f_W2, f_b2, f_W3, f_b3)
    return (Xr * group_mask).sum(axis=1).astype(np.float32)


def kernel(X, W, embeddings, g_W1, g_b1, g_W2, g_b2, g_W3, g_b3,
           f_W1, f_b1, f_W2, f_b2, f_W3, f_b3, group_mask, _run_kw=None):
    if not np.allclose(group_mask, np.eye(N, D, dtype=np.float32)):
        return _kernel_numpy(X, W, embeddings, g_W1, g_b1, g_W2, g_b2, g_W3,
                             g_b3, f_W1, f_b1, f_W2, f_b2, f_W3, f_b3,
                             group_mask)

    from concourse import bass_utils

    consts = _host_consts(W, embeddings, g_W1, g_b1, g_W2, g_b2, g_W3, g_b3,
                          f_W1, f_b1, f_W2, f_b2, f_W3, f_b3)
    XTB = _bf16(np.asarray(X, np.float32).T)  # [N, B]
    in_maps = []
    for k in range(NCORES):
        m = dict(consts)
        m["XTB"] = np.ascontiguousarray(XTB[:, k * BC : (k + 1) * BC])
        in_maps.append(m)

    nc = _get_program()
    res = bass_utils.run_bass_kernel_spmd(
        nc, in_maps, core_ids=list(range(NCORES)), **(_run_kw or {})
    )
    out = np.empty((B, D), np.float32)
    for k in range(NCORES):
        out[k * BC : (k + 1) * BC, :] = res.results[k]["OUT"].T
    out += f_b3.reshape(1, D).astype(np.float32)
    if _run_kw:
        kernel.last_results = res
    return out


# revision 112
# speedup vs baseline: 1.5173x; 1.0008x over previous
"""Trainium2 Bass kernel for ContractiveInvertibleGNN feed-forward.

Math (reference, with group_mask == I_32):
  out[b,i] = f_i( sum_j W_adj[j,i] * g_j(X[b,j]) )
where g_j: R -> R^32 and f_i: R^32 -> R are slices of two shared MLPs
(64->128->128->32 with a residual middle block, LeakyReLU 0.01):
  g: H1 = lrelu(X[b,j]*U_j + C1_j); H2 = H1 + lrelu(H1@W2g + b2g)
     X_emb = H2 @ W3g            (g_b3 folded into C2)
  f: Hf1 = lrelu(X_aggr@Wf1x + C2_i)
     Hf2 = Hf1 + lrelu(Hf1@Wf2 + bf2); out_i = Hf2 . V_i (+ f_b3_i on host)

Sharding: pure data-parallel over batch across 8 cores (2048 rows each).

Per-core schedule: batch is processed in 2 halves of 1024, emitted as a
software pipeline (per-engine execution follows emission order):
g(h0); then g(h1) interleaved with the agg of h0 (T1 transpose ->
block-diag matmul -> T2 transpose) and the first f(h0) nodes; then the
rest of f(h0) interleaved with agg(h1); then f(h1). gw3 / V-dot matmuls
are emitted with a 1-2 node skew so the PE never waits on the current
node's activations.

Most SBUF interchange tiles are bf16 (DVE 2x/4x modes, 1 cyc/row PE,
half DMA); PSUM stays f32. HW constraints honored: GPSIMD never touches
PSUM; StreamTranspose src/dst dtypes match (so Xa stays f32 and is
re-rounded to f32r for the PE by small SBUF->SBUF DMA hops); no mixed
32/16-bit matmul inputs. Engine assignment: Act = psum-sourced lrelus
(t2/hf1/tf) + a per-phase share of h1; DVE = most of the h1 lrelu
(tensor_scalar 4x path), transposes, psum->bf16 copies, tf tail, and a
slice of the f-residual add; Pool = SBUF-only work (h1 tail, most of
the f-residual add); PE = matmuls with the g-residual folded as double
accumulating matmuls. Final dot V_i routes node i / quarter c to PSUM
partition 4i+c via a padded stationary table (VPX), giving one
[128, 256] output copy + one DMA per half.
"""

import os
import sys

import numpy as np

for _p in ("/opt/trn_rl_repo", "/root/.axon_site/_ro/trn_rl_repo"):
    if os.path.isdir(_p) and _p not in sys.path:
        sys.path.insert(0, _p)

N = 32          # nodes
D = 32          # processed dim (== N, group_mask = I)
A = 128         # hidden width
B = 16384       # batch
NCORES = 8
BC = B // NCORES        # 2048 rows per core
NH = 2                  # batch halves per core
BH = BC // NH           # 1024 rows per half
NCH = 4                 # partition-group (quarter) stacking factor
CH = BH // NCH          # 256 cols per quarter
ALPHA = 0.01

# knobs
TG_POOL = 0             # t2 cols per half on Pool engine (rest on Act)
PF_DVE = 384            # tf cols per half on DVE (rest on Act)
VREG = A + 3            # per-node stationary region width in VPX


def _build_program():
    from contextlib import ExitStack

    from concourse import bacc, bass, mybir, tile

    f32 = mybir.dt.float32
    f32r = mybir.dt.float32r
    bf16 = mybir.dt.bfloat16
    LRELU = mybir.ActivationFunctionType.Lrelu
    ALU_MULT = mybir.AluOpType.mult
    ALU_ADD = mybir.AluOpType.add
    ALU_MAX = mybir.AluOpType.max

    nc = bacc.Bacc("TRN2", target_bir_lowering=False, debug=False)

    def din(name, shape, dt):
        return nc.dram_tensor(
            name, list(shape), dt, kind="ExternalInput"
        ).ap()

    xtb_d = din("XTB", (N, BC), bf16)
    gw2_d = din("GW2", (A, A), bf16)
    fw2_d = din("FW2", (A, A), bf16)
    gw3p_d = din("GW3P", (A, NCH * A), bf16)   # col-block c: g_W3 at cols 32c..
    fw1p_d = din("FW1P", (A, NCH * A), bf16)   # row-block c: f_W1[:32] rows 32c
    bd_d = din("BD", (A, A), bf16)             # kron(I4, W_adj)
    vpx_d = din("VPX", (A, N * VREG), bf16)    # region i: V_i at col 131i+4i+3
    u_d = din("U", (A, N), f32)
    ua_d = din("UA", (A, N), f32)
    c1_d = din("C1", (A, N), f32)
    c1a_d = din("C1A", (A, N), f32)
    c2_d = din("C2", (A, N), f32)
    gb2_d = din("GB2", (A, 1), f32)
    fb2_d = din("FB2", (A, 1), f32)
    out_d = nc.dram_tensor("OUT", [N, BC], f32, kind="ExternalOutput").ap()

    with tile.TileContext(nc) as tc, ExitStack() as ctx:
        const = ctx.enter_context(tc.tile_pool(name="const", bufs=1))
        bigp = ctx.enter_context(tc.tile_pool(name="big", bufs=2))
        workp = ctx.enter_context(tc.tile_pool(name="work", bufs=5))
        xbcp = ctx.enter_context(tc.tile_pool(name="xbc", bufs=5))
        lrp = ctx.enter_context(tc.tile_pool(name="lrp", bufs=2))
        outp = ctx.enter_context(tc.tile_pool(name="outs", bufs=2))
        xarp = ctx.enter_context(tc.tile_pool(name="xar", bufs=3))
        # PSUM bank budget (8 banks of 2KB): ppG 3 + ppF 4 + ppV 1.
        # ppG serves gw2-out, gw3-out pairs and agg windows; ppF serves
        # the f phase only, so g(h+1) never waits on f(h) ring slots.
        ppG = ctx.enter_context(tc.tile_pool(name="ppG", bufs=3, space="PSUM"))
        ppF = ctx.enter_context(tc.tile_pool(name="ppF", bufs=4, space="PSUM"))
        ppV = ctx.enter_context(tc.tile_pool(name="ppV", bufs=1, space="PSUM"))

        def load_const(ap_dram, shape):
            t = const.tile(list(shape), ap_dram.dtype,
                           tag=f"c_{ap_dram.tensor.name}")
            nc.sync.dma_start(t[:, :], ap_dram)
            return t

        st = [dict() for _ in range(NH)]
        u_s = load_const(u_d, (A, N))
        ua_s = load_const(ua_d, (A, N))
        c1_s = load_const(c1_d, (A, N))
        c1a_s = load_const(c1a_d, (A, N))
        gw2_s = load_const(gw2_d, (A, A))
        gb2_s = load_const(gb2_d, (A, 1))
        # prefetch the first broadcast rows ahead of the heavy tables so
        # compute starts immediately; heavy consts are first needed at
        # the j=3 X_emb projection.
        st[0]["pre"] = {}
        for _pj in range(4):
            _xpre = xbcp.tile([A, BH], bf16, tag="xbc", name="xbc_pre")
            nc.sync.dma_start(
                _xpre[:, :],
                xtb_d[_pj : _pj + 1, 0:BH].partition_broadcast(A),
            )
            st[0]["pre"][_pj] = _xpre
        heavy = {}
        heavy["gw3p"] = load_const(gw3p_d, (A, NCH * A))
        heavy["bd"] = load_const(bd_d, (A, A))
        heavy["fw1p"] = load_const(fw1p_d, (A, NCH * A))
        heavy["fw2"] = load_const(fw2_d, (A, A))
        heavy["c2"] = load_const(c2_d, (A, N))
        heavy["fb2"] = load_const(fb2_d, (A, 1))

        # Per-half state for the software-pipelined emission below.
        TSP = BH - TG_POOL
        SPL = BH - PF_DVE
        TS = 8                  # T1 transpose steps per half
        WT = 512 // D           # 16 t per agg window
        NW = CH * D // 512      # 16 agg windows per half
        def g_node(h, j, h1_act=False, gpool=None):
            """Emit g work for node j; gw3+copy are skewed (emitted for
            j-1) so the PE never stalls on the current node's t2. The h1
            lrelu runs on Act or DVE per the h1_act flag, chosen by the
            emission schedule to balance per-phase engine load."""
            s = st[h]
            if j == 0:
                s["xe"] = bigp.tile([A, N * CH], bf16, tag="xe", name="xe")
                s["gq"] = {}
            if j in s.get("pre", {}):
                xbc = s["pre"].pop(j)
            else:
                xbc = xbcp.tile([A, BH], bf16, tag="xbc")
                nc.sync.dma_start(
                    xbc[:, :],
                    xtb_d[j : j + 1,
                          h * BH : (h + 1) * BH].partition_broadcast(A),
                )
            h1 = workp.tile([A, BH], bf16, tag="h1")
            uj = u_s[:, j : j + 1]
            cj = c1_s[:, j : j + 1]
            uaj = ua_s[:, j : j + 1]
            caj = c1a_s[:, j : j + 1]
            if h1_act:
                nc.scalar.activation(h1[:, :], xbc[:, :], LRELU,
                                     bias=cj, scale=uj, alpha=ALPHA)
            else:
                GP = 384  # h1 tail cols on the otherwise-idle Pool engine
                zt = lrp.tile([A, BH], bf16, tag="z")
                mt = lrp.tile([A, BH], bf16, tag="m")
                nc.vector.tensor_scalar(zt[:, : BH - GP], xbc[:, : BH - GP],
                                        uj, cj, ALU_MULT, ALU_ADD)
                nc.vector.tensor_scalar(mt[:, : BH - GP], xbc[:, : BH - GP],
                                        uaj, caj, ALU_MULT, ALU_ADD)
                nc.vector.tensor_tensor(h1[:, : BH - GP], zt[:, : BH - GP],
                                        mt[:, : BH - GP], ALU_MAX)
                nc.gpsimd.tensor_scalar(zt[:, BH - GP :], xbc[:, BH - GP :],
                                        uj, cj, ALU_MULT, ALU_ADD)
                nc.gpsimd.tensor_scalar(mt[:, BH - GP :], xbc[:, BH - GP :],
                                        uaj, caj, ALU_MULT, ALU_ADD)
                nc.vector.tensor_tensor(h1[:, BH - GP :], zt[:, BH - GP :],
                                        mt[:, BH - GP :], ALU_MAX)
            # middle residual block: t2 = lrelu(h1 @ gw2 + gb2)
            t2 = workp.tile([A, BH], bf16, tag="t2")
            for q in range(2):
                pa = (gpool or ppG).tile(
                    [A, 512], f32,
                    tag="pF" if gpool is ppF else "pG", name="pa")
                sl = slice(q * 512, (q + 1) * 512)
                nc.tensor.matmul(pa[:, :], gw2_s[:, :], h1[:, sl],
                                 start=True, stop=True)
                if q == 0:
                    nc.scalar.activation(t2[:, sl], pa[:, :], LRELU,
                                         bias=gb2_s[:, 0:1], alpha=ALPHA)
                else:
                    nc.scalar.activation(
                        t2[:, 512 : TSP], pa[:, : TSP - 512], LRELU,
                        bias=gb2_s[:, 0:1], alpha=ALPHA,
                    )
                    if TG_POOL:
                        gz = lrp.tile([A, TG_POOL], bf16, tag="gz")
                        nc.gpsimd.tensor_scalar(gz[:, :], pa[:, TSP - 512 :],
                                                ALPHA, None, ALU_MULT)
                        nc.gpsimd.tensor_tensor(t2[:, TSP:],
                                                pa[:, TSP - 512 :],
                                                gz[:, :], ALU_MAX)
            s["gq"][j] = (h1, t2)
            # skew: emit X_emb projection for the pair (j-3, j-2) so the
            # PE never stalls on the current node's t2.
            if j >= 3 and j % 2 == 1:
                g_emb(h, j - 3)

        def g_emb(h, j):
            """X_emb for nodes j and j+1 into one PSUM tile: gw3^T @
            (h1 + t2) with the residual folded as double accumulating
            matmuls; quarter c routed to psum rows 32c."""
            s = st[h]
            pm3_full = ppG.tile([A, 512], f32, tag="pG", name="pm3")
            for u in range(2):
                h1, t2 = s["gq"].pop(j + u)
                pm3 = pm3_full[:, u * CH : (u + 1) * CH]
                for c in range(NCH):
                    lt = heavy["gw3p"][:, c * A : (c + 1) * A]
                    csl = slice(c * CH, (c + 1) * CH)
                    nc.tensor.matmul(pm3[:, :], lt, h1[:, csl],
                                     start=(c == 0), stop=False)
                    nc.tensor.matmul(pm3[:, :], lt, t2[:, csl],
                                     start=False, stop=(c == NCH - 1))
            nc.vector.tensor_copy(s["xe"][:, j * CH : (j + 2) * CH],
                                  pm3_full[:, :])

        def agg_step(h, k):
            """Step k of the aggregation pipeline: 0..TS-1 are T1 block
            transposes, TS..TS+NW-1 are (block-diag matmul, T2) pairs."""
            s = st[h]
            if k == 0:
                s["xtile"] = bigp.tile([A, CH * D], bf16, tag="xt",
                                       name="xtile", bufs=1)
                s["xa"] = bigp.tile([A, N * CH], f32, tag="xa", name="xa")
            xe, xtile, xa = s["xe"], s["xtile"], s["xa"]
            if k < TS:
                xt3 = xe.rearrange("p (j t) -> p j t",
                                   j=N).transpose([0, 2, 1])
                xto = xtile.rearrange("p (t d) -> p t d", d=D)
                tstep = CH // TS
                nc.vector.transpose(
                    xto[:, k * tstep : (k + 1) * tstep, :],
                    xt3[:, k * tstep : (k + 1) * tstep, :],
                )
                return
            w = k - TS
            xa3 = xa.rearrange("p (i t) -> p i t", i=N).transpose([0, 2, 1])
            pg = ppG.tile([A, 512], f32, tag="pG", name="pg")
            nc.tensor.matmul(
                pg[:, :], heavy["bd"][:, :],
                xtile[:, w * 512 : (w + 1) * 512], start=True, stop=True,
            )
            nc.vector.transpose(
                xa3[:, w * WT : (w + 1) * WT, :],
                pg.rearrange("p (t d) -> p t d", d=D)[:, :, :],
            )

        def f_hop(h, i):
            # rounded-bits SBUF->SBUF DMA: xa (f32) -> f32r for the PE,
            # two nodes per transfer
            s = st[h]
            xr = xarp.tile([A, 2 * CH], f32r, tag="xr", name="xr")
            nc.sync.dma_start(
                xr[:, :], s["xa"].bitcast(f32r)[:, i * CH : (i + 2) * CH])
            s["hop"][i] = xr

        def f_node(h, i):
            """Emit f work for node i; the V-dot is skewed (emitted for
            i-1) so the PE never stalls on the current node's hf2."""
            s = st[h]
            if i == 0:
                s["vps"] = ppV.tile([A, CH], f32, tag="pV", name="vps")
                s["fq"] = {}
                s["hop"] = {}
                f_hop(h, 0)
                f_hop(h, 2)
                f_hop(h, 4)
            if i % 2 == 0 and i + 6 < N:
                f_hop(h, i + 6)
            xr = s["hop"].pop(i - 1) if i % 2 else s["hop"][i]
            rhs = xr[:, (i % 2) * CH : (i % 2 + 1) * CH]
            hf1 = workp.tile([A, BH], bf16, tag="hf1", bufs=3)
            for q in range(2):
                paf = ppF.tile([A, 512], f32, tag="pF", name="paf")
                for cc in range(2):
                    c = 2 * q + cc
                    nc.tensor.matmul(
                        paf[:, cc * CH : (cc + 1) * CH],
                        heavy["fw1p"][:, c * A : (c + 1) * A], rhs,
                        start=True, stop=True,
                    )
                sl = slice(q * 512, (q + 1) * 512)
                nc.scalar.activation(hf1[:, sl], paf[:, :], LRELU,
                                     bias=heavy["c2"][:, i : i + 1],
                                     alpha=ALPHA)
                # skew: V-dots from three nodes back fill the PE while
                # this node's activations run.
                if q == 0 and i > 2:
                    f_vdot(h, i - 3)
            tf = workp.tile([A, BH], bf16, tag="tf", bufs=3)
            for q in range(2):
                pbf = ppF.tile([A, 512], f32, tag="pF", name="pbf")
                sl = slice(q * 512, (q + 1) * 512)
                for cc in range(2):
                    c = 2 * q + cc
                    csl = slice(c * CH, (c + 1) * CH)
                    nc.tensor.matmul(pbf[:, cc * CH : (cc + 1) * CH],
                                     heavy["fw2"][:, :], hf1[:, csl],
                                     start=True, stop=True)
                if q == 0:
                    nc.scalar.activation(tf[:, sl], pbf[:, :], LRELU,
                                         bias=heavy["fb2"][:, 0:1],
                                         alpha=ALPHA)
                else:
                    asl = 512 - (PF_DVE if h == 0 else 512)
                    if asl:
                        nc.scalar.activation(tf[:, 512 : 512 + asl],
                                             pbf[:, :asl], LRELU,
                                             bias=heavy["fb2"][:, 0:1],
                                             alpha=ALPHA)
                    dz = lrp.tile([A, 512], bf16, tag="dz")
                    dz = dz[:, : 512 - asl]
                    nc.vector.tensor_scalar(dz[:, :], pbf[:, asl:],
                                            ALPHA, None, ALU_MULT)
                    nc.vector.tensor_tensor(tf[:, 512 + asl :],
                                            pbf[:, asl:], dz[:, :],
                                            ALU_MAX)
            hf2 = workp.tile([A, BH], bf16, tag="hf2", bufs=5)
            # residual add split across the SBUF-only Pool engine and DVE
            nc.gpsimd.tensor_tensor(hf2[:, :640], hf1[:, :640],
                                    tf[:, :640], ALU_ADD)
            nc.vector.tensor_tensor(hf2[:, 640:], hf1[:, 640:],
                                    tf[:, 640:], ALU_ADD)
            s["fq"][i] = hf2

        def f_vdot(h, i):
            # out_i = hf2 . V_i via accumulating matmuls; stationary
            # window puts V_i at psum partition 4i+c.
            s = st[h]
            hf2 = s["fq"].pop(i)
            for c in range(NCH):
                base = VREG * i + 3 - c
                lt = heavy["vpx"][:, base : base + A]
                sl = slice(c * CH, (c + 1) * CH)
                nc.tensor.matmul(
                    s["vps"][:, :], lt, hf2[:, sl],
                    start=(i == 0 and c == 0),
                    stop=(i == N - 1 and c == NCH - 1),
                )

        def f_out(h):
            f_vdot(h, N - 3)
            f_vdot(h, N - 2)
            f_vdot(h, N - 1)
            osb = outp.tile([A, CH], f32, tag="o")
            nc.vector.tensor_copy(osb[:, :], st[h]["vps"][:, :])
            nc.gpsimd.dma_start(
                out_d[:, h * BH : (h + 1) * BH].rearrange(
                    "i (c t) -> i c t", c=NCH),
                osb[:, :],
            )

        # ---- software-pipelined emission across the two halves ----
        NAGG = TS + NW  # 20 agg steps per half
        for j in range(N):
            g_node(0, j, h1_act=(j % 8 == 7), gpool=ppF)
            if j == 0:
                heavy["vpx"] = load_const(vpx_d, (A, N * VREG))
        g_emb(0, N - 2)
        # agg(0) interleaved with the first 20 nodes of g(1); the last 12
        # nodes of g(1) interleave 1:1 with the first 12 nodes of f(0).
        fi = 0
        for k in range(N):
            g_node(1, k, h1_act=(k < NAGG and k % 3 == 0),
                   gpool=(ppF if k < NAGG - 2 else None))
            if k < NAGG:
                agg_step(0, k)
            else:
                f_node(0, fi)
                fi += 1
        g_emb(1, N - 2)
        # remaining f(0) nodes interleave with agg(1)
        for k in range(NAGG):
            if fi < N:
                f_node(0, fi)
                fi += 1
            agg_step(1, k)
        while fi < N:
            f_node(0, fi)
            fi += 1
        f_out(0)
        for i in range(N):
            f_node(1, i)
        f_out(1)

    nc.compile()
    return nc


_NC_CACHE = {}


def _get_program():
    if "nc" not in _NC_CACHE:
        _NC_CACHE["nc"] = _build_program()
    return _NC_CACHE["nc"]


def _bf16(x):
    import ml_dtypes

    return np.ascontiguousarray(np.asarray(x, np.float32).astype(
        ml_dtypes.bfloat16))


def _host_consts(W, embeddings, g_W1, g_b1, g_W2, g_b2, g_W3, g_b3,
                 f_W1, f_b1, f_W2, f_b2, f_W3, f_b3):
    f = np.float32
    W_adj = (W * (1.0 - np.eye(N, dtype=f))).astype(f)
    U = np.ascontiguousarray(g_W1[:D].T, dtype=f)                    # [A, N]
    C1 = np.ascontiguousarray((embeddings @ g_W1[D:] + g_b1).T, f)   # [A, N]
    s = W_adj.sum(axis=0)                                            # [N]
    C2 = (embeddings @ f_W1[D:] + f_b1 + np.outer(s, g_b3 @ f_W1[:D]))
    C2 = np.ascontiguousarray(C2.T, dtype=f)                         # [A, N]
    GW3P = np.zeros((A, NCH * A), f)
    FW1P = np.zeros((A, NCH * A), f)
    for c in range(NCH):
        GW3P[:, c * A + c * D : c * A + (c + 1) * D] = g_W3
        FW1P[c * D : (c + 1) * D, c * A : (c + 1) * A] = f_W1[:D]
    BD = np.kron(np.eye(NCH, dtype=f), W_adj).astype(f)
    VPX = np.zeros((A, N * VREG), f)
    for i in range(N):
        VPX[:, VREG * i + 4 * i + 3] = f_W3[:, i]
    return {
        "GW2": _bf16(g_W2),
        "FW2": _bf16(f_W2),
        "GW3P": _bf16(GW3P), "FW1P": _bf16(FW1P), "BD": _bf16(BD),
        "VPX": _bf16(VPX),
        "U": U, "UA": (ALPHA * U).astype(f), "C1": C1,
        "C1A": (ALPHA * C1).astype(f), "C2": C2,
        "GB2": np.ascontiguousarray(g_b2.reshape(A, 1), f),
        "FB2": np.ascontiguousarray(f_b2.reshape(A, 1), f),
    }


def _kernel_numpy(X, W, embeddings, g_W1, g_b1, g_W2, g_b2, g_W3, g_b3,
                  f_W1, f_b1, f_W2, f_b2, f_W3, f_b3, group_mask):
    # general fallback (non-identity group_mask)
    def lrelu(x):
        return np.where(x > 0, x, ALPHA * x)

    def mlp(x, W1, b1, W2, b2, W3, b3):
        h = lrelu(x @ W1 + b1)
        h = h + lrelu(h @ W2 + b2)
        return h @ W3 + b3

    n = W.shape[0]
    W_adj = W * (1.0 - np.eye(n, dtype=W.dtype))
    Xm = X[:, None, :] * group_mask
    E = np.broadcast_to(embeddings, (X.shape[0], n, embeddings.shape[1]))
    Xe = mlp(np.concatenate([Xm, E], 2), g_W1, g_b1, g_W2, g_b2, g_W3, g_b3)
    Xa = np.einsum("ji,bjd->bid", W_adj, Xe)
    Xr = mlp(np.concatenate([Xa, E], 2), f_W1, f_b1, f_W2, f_b2, f_W3, f_b3)
    return (Xr * group_mask).sum(axis=1).astype(np.float32)


def kernel(X, W, embeddings, g_W1, g_b1, g_W2, g_b2, g_W3, g_b3,
           f_W1, f_b1, f_W2, f_b2, f_W3, f_b3, group_mask, _run_kw=None):
    if not np.allclose(group_mask, np.eye(N, D, dtype=np.float32)):
        return _kernel_numpy(X, W, embeddings, g_W1, g_b1, g_W2, g_b2, g_W3,
                             g_b3, f_W1, f_b1, f_W2, f_b2, f_W3, f_b3,
                             group_mask)

    from concourse import bass_utils

    consts = _host_consts(W, embeddings, g_W1, g_b1, g_W2, g_b2, g_W3, g_b3,
                          f_W1, f_b1, f_W2, f_b2, f_W3, f_b3)
    XTB = _bf16(np.asarray(X, np.float32).T)  # [N, B]
    in_maps = []
    for k in range(NCORES):
        m = dict(consts)
        m["XTB"] = np.ascontiguousarray(XTB[:, k * BC : (k + 1) * BC])
        in_maps.append(m)

    nc = _get_program()
    res = bass_utils.run_bass_kernel_spmd(
        nc, in_maps, core_ids=list(range(NCORES)), **(_run_kw or {})
    )
    out = np.empty((B, D), np.float32)
    for k in range(NCORES):
        out[k * BC : (k + 1) * BC, :] = res.results[k]["OUT"].T
    out += f_b3.reshape(1, D).astype(np.float32)
    if _run_kw:
        kernel.last_results = res
    return out
